# revision 1
# baseline (speedup 1.0000x reference)
"""Trainium2 Bass kernel for the 3-expert MoE routing MLP.

Reference computation (B=1M rows):
    y1  = tanh(x @ w1 - b1)                     # [B, 8]
    h_k = sigmoid(y1 @ wa_k - ba_k)             # [B, 16] for experts k=0,1,2
    e_k = h_k @ wb_k - bb_k                     # [B, 32]
    y   = e_{u[b]}  per row b

Device mapping (pure data parallel over 8 cores, B_C = 125000 rows/core):

  * Host packs each core's x shard transposed, two row-blocks deep:
    x2 [128, B_H] fp16 with partitions 0-63 = x[:B_H].T, 64-127 = x[B_H:].T,
    so every PE matmul streams two batch halves at once (B_H = B_C/2).

  * The whole per-row pipeline is 4 fp16 matmuls + ONE sigmoid + one
    int-compare + one PSUM->SBUF copy, using a software-pipelined
    ("skewed") PSUM bank S(c) per 500-column chunk c:

      rows 0-15  : 2*(x@w1 - b1)(c)        <- mmA (lhsT 2*w1 blocks)
      rows 16-21 : BIG*onehot(c)           <- mmB (from DVE is_equal)
      rows 22-27 : BIG*pseudo-onehot(c-1)  <- mmH (propagated)
      rows 32-127: H-preact(c-1) + BIG*mask<- mmH (from T(c-1) rows 0:22)

    One ACT sigmoid over S(c) then yields T(c) fp16:
      rows 0-15  : y1t'(c)   = sigmoid(2*(x@w1-b1))      (tanh = 2*sig-1)
      rows 16-21 : poh(c)    ~= onehot(c)    (sigmoid saturation)
      rows 22-27 : poh(c-1)  ~= onehot(c-1)
      rows 32-127: G(c-1)    = sigmoid(P_h - ba - BIG(1-mask)) ~= mask*h

    mmF(c-1) then computes the final output in one stream from T(c):
      lhsT rows 22-27 = -bb_k rows (bias select via poh(c-1)),
      lhsT rows 32-127 = block-diag wb (the tanh "-1" constant is folded
      into the sigmoid bias as -colsum(wa)).

  * Output is written transposed ([64, B_H] f32 per core) for contiguous
    DMA; the host unpacks back to [B, 32].
"""

import math

import numpy as np

import concourse.bass as bass
import concourse.tile as tile
from concourse import mybir
from concourse.bass_utils import run_bass_kernel_spmd

F32 = mybir.dt.float32
F16 = mybir.dt.float16
I32 = mybir.dt.int32

N_CORES = 8
B = 1_000_000
IN = 64
OUT = 32
B_C = B // N_CORES          # rows per core
B_H = B_C // 2              # packed free length per core
BIG = 28.0                  # saturation constant (sigmoid(-14) ~ 8e-7)

F_TILE = 2000               # SBUF tile free size (4 chunks)
CHUNK = 500                 # PSUM matmul free size (<=512 fp32 out)


def _pack_weights(w1, b1, w2, b2, w3, b3, w4, b4, w5, b5, w6, b6, w7, b7):
    f32 = np.float32
    # mmA lhsT [128, 16]: 2*w1 block-diag (tanh(z) = 2*sigmoid(2z) - 1).
    W_a = np.zeros((128, 16), f32)
    W_a[0:64, 0:8] = 2.0 * w1
    W_a[64:128, 8:16] = 2.0 * w1

    # mmH lhsT [22, 96]: K-rows = T_y[0:22] = [y1t'(16); onehot(6)].
    #   y1t' rows: 2*wa blocks (y1 = 2*y1t' - 1)
    #   onehot rows: +BIG on the selected expert's 16 hidden rows
    wa_all = np.concatenate([w2, w4, w6], axis=1)        # [8, 48]
    W_h = np.zeros((22, 96), f32)
    W_h[0:8, 0:48] = 2.0 * wa_all
    W_h[8:16, 48:96] = 2.0 * wa_all
    for k in range(3):
        W_h[16 + k, 16 * k:16 * (k + 1)] = BIG
        W_h[19 + k, 48 + 16 * k:64 + 16 * k] = BIG

    # mmF lhsT [102, 64]: rows 0-95 wb blocks, rows 96-101 -bb (onehot).
    wb_all = np.concatenate([w3, w5, w7], axis=0)        # [48, 32]
    bb = [b3, b5, b7]
    W_f = np.zeros((102, 64), f32)
    W_f[0:48, 0:32] = wb_all
    W_f[48:96, 32:64] = wb_all
    for k in range(3):
        W_f[96 + k, 0:32] = -bb[k]
        W_f[99 + k, 32:64] = -bb[k]

    # sigmoid biases: trunk -2*b1 ; H: -ba - BIG - colsum(wa) (the "-1"
    # of y1 = 2*y1t' - 1 folded in).
    ba_all = np.concatenate([b2, b4, b6])                # [48]
    colsum = wa_all.sum(axis=0)                          # [48]
    b1v = np.concatenate([-2.0 * b1, -2.0 * b1]).astype(f32)
    b2v = np.concatenate([-ba_all - BIG - colsum] * 2).astype(f32)

    # bpack [102, 3]: col0 rows0-15 = b1v, col1 rows0-95 = b2v,
    # col2 rows0-5 = kvec.
    bpack = np.zeros((102, 3), f32)
    bpack[0:16, 0] = b1v
    bpack[0:96, 1] = b2v
    bpack[0:6, 2] = np.array([0, 1, 2, 0, 1, 2], f32)

    # wpack fp16 [128, 176]: cols 0:16 W_a, 16:112 W_h, 112:176 W_f.
    wpack = np.zeros((128, 176), np.float16)
    wpack[:, 0:16] = W_a.astype(np.float16)
    wpack[0:22, 16:112] = W_h.astype(np.float16)
    wpack[0:102, 112:176] = W_f.astype(np.float16)
    return dict(wpack=wpack, bpack=bpack)


def _split_multi_waits(nc):
    """Walrus codegen allows one sync-wait per instruction; hoist extra
    waits onto same-engine NoOps inserted just before the instruction."""
    n = 0
    for fn in nc.m.functions:
        for blk in fn.blocks:
            out = []
            for ins in blk.instructions:
                si = ins.sync_info
                if si is not None and len(si.on_wait) > 1:
                    waits = list(si.on_wait)
                    for j, w in enumerate(waits[:-1]):
                        nop = mybir.InstNoOp(name=f"{ins.name}-wsplit{j}")
                        nop.engine = ins.engine
                        nop.sync_info = mybir.SyncInfo(on_wait=[w],
                                                       on_update=[])
                        nc.register_instruction(nop)
                        out.append(nop)
                        n += 1
                    si.on_wait = [waits[-1]]
                out.append(ins)
            blk.instructions[:] = out
    return n


def build_nc(b_h=B_H, f_tile=F_TILE, chunk=CHUNK):
    nc = bass.Bass("TRN2", target_bir_lowering=False, debug=False)

    x2_d = nc.dram_tensor("x2", [128, b_h], F16, kind="ExternalInput").ap()
    u6_d = nc.dram_tensor("u6", [6, b_h], I32, kind="ExternalInput").ap()
    wp_d = nc.dram_tensor("wpack", [128, 176], F16, kind="ExternalInput").ap()
    bp_d = nc.dram_tensor("bpack", [102, 3], F32, kind="ExternalInput").ap()
    yt_d = nc.dram_tensor("yT", [64, b_h], F32, kind="ExternalOutput").ap()

    assert b_h % chunk == 0 and f_tile % chunk == 0
    n_tiles = math.ceil(b_h / f_tile)

    SIG = mybir.ActivationFunctionType.Sigmoid

    with tile.TileContext(nc) as tc:
        with (
            tc.tile_pool(name="const", bufs=1) as cpool,
            tc.tile_pool(name="xin", bufs=4) as xpool,
            tc.tile_pool(name="uin", bufs=3) as upool,
            tc.tile_pool(name="toh", bufs=2) as ohpool,
            tc.tile_pool(name="ty", bufs=2) as ypool,
            tc.tile_pool(name="tg", bufs=2) as gpool,
            tc.tile_pool(name="outp", bufs=3) as opool,
            tc.tile_pool(name="st", bufs=2, space="PSUM") as stp,
            tc.tile_pool(name="sh", bufs=3, space="PSUM") as shp,
            tc.tile_pool(name="pfp", bufs=3, space="PSUM") as pfp,
        ):
            wp = cpool.tile([128, 176], F16)
            nc.sync.dma_start(wp[:], wp_d)
            W_a = wp[:, 0:16]
            W_h = wp[0:22, 16:112]
            W_f = wp[0:102, 112:176]
            bp = cpool.tile([102, 3], F32)
            nc.sync.dma_start(bp[:], bp_d)
            b1v = bp[0:16, 0:1]
            b2v = bp[0:96, 1:2]
            kv = bp[0:6, 2:3]

            # HAM warmup: ~8us of dependency-free back-to-back matmuls so
            # the PE clock gate opens (4/8 -> 8/8) before the main loop.
            for _ in range(28):
                wps = stp.tile([16, chunk], F32, name="wps", tag="S_t")
                nc.tensor.matmul(wps[:, 0:176], wp[:, 0:16], wp[:, 0:176],
                                 start=True, stop=True)

            n_chunks = b_h // chunk
            xt = {}; ut = {}; oh = {}; T_y = {}; T_g = {}; ot = {}
            S_t = {}; S_h = {}; p_f = {}
            cpt = f_tile // chunk

            def colsl(c):
                return slice((c % cpt) * chunk, (c % cpt) * chunk + chunk)

            # Software-pipelined emission: per step s the PE does
            # mmA(s), mmH(s-1), mmF(s-2) so every PE wait is pre-satisfied
            # and the PE stream stays dense.
            for s in range(n_chunks + 2):
                c0, c1, c2 = s, s - 1, s - 2
                if c0 < n_chunks:
                    t = c0 // cpt
                    if c0 % cpt == 0:
                        f0 = t * f_tile
                        fs = min(f_tile, b_h - f0)
                        xt[t] = xpool.tile([128, f_tile], F16, name="xt")
                        nc.sync.dma_start(xt[t][:, :fs], x2_d[:, f0:f0 + fs])
                        ut[t] = upool.tile([6, f_tile], I32, name="ut")
                        nc.sync.dma_start(ut[t][:, :fs], u6_d[:, f0:f0 + fs])
                        oh[t] = ohpool.tile([6, f_tile], F16, name="oh")
                        nc.vector.tensor_scalar(
                            oh[t][:, :fs], ut[t][:, :fs], kv[:], None,
                            mybir.AluOpType.is_equal)
                        T_y[t] = ypool.tile([22, f_tile], F16, name="T_y")
                        T_g[t] = gpool.tile([102, f_tile], F16, name="T_g")
                        nc.sync.dma_start(T_y[t][16:22, :fs], oh[t][:, :fs])
                        nc.sync.dma_start(T_g[t][96:102, :fs], oh[t][:, :fs])
                        ot[t] = opool.tile([64, f_tile], F32, name="ot")
                    cc = colsl(c0)
                    S_t[c0] = stp.tile([16, chunk], F32, name="S_t")
                    nc.tensor.matmul(S_t[c0][:], W_a, xt[t][:, cc],
                                     start=True, stop=True)
                    nc.scalar.activation(T_y[t][0:16, cc], S_t[c0][:], SIG,
                                         bias=b1v[:], scale=1.0)
                    del S_t[c0]
                if 0 <= c2 < n_chunks:
                    t2 = c2 // cpt
                    cc = colsl(c2)
                    pf = pfp.tile([64, chunk], F32, name="pf")
                    nc.tensor.matmul(pf[:], W_f, T_g[t2][0:102, cc],
                                     start=True, stop=True)
                    nc.vector.tensor_copy(ot[t2][:, cc], pf[:])
                    if c2 % cpt == cpt - 1 or c2 == n_chunks - 1:
                        f0 = t2 * f_tile
                        ofs = min(f_tile, b_h - f0)
                        nc.gpsimd.dma_start(yt_d[:, f0:f0 + ofs],
                                            ot[t2][:, :ofs])
                if 0 <= c1 < n_chunks:
                    t1 = c1 // cpt
                    cc = colsl(c1)
                    S_h[c1] = shp.tile([96, chunk], F32, name="S_h")
                    nc.tensor.matmul(S_h[c1][:], W_h, T_y[t1][0:22, cc],
                                     start=True, stop=True)
                    nc.scalar.activation(T_g[t1][0:96, cc], S_h[c1][:], SIG,
                                         bias=b2v[:], scale=1.0)
                    del S_h[c1]

    _split_multi_waits(nc)
    return nc


_NC_CACHE = {}


def _get_nc(b_h=B_H, f_tile=F_TILE, chunk=CHUNK):
    key = (b_h, f_tile, chunk)
    if key not in _NC_CACHE:
        _NC_CACHE[key] = build_nc(*key)
    return _NC_CACHE[key]


def make_in_maps(x, u, weights, n_cores=N_CORES):
    """Shard + pack full inputs into per-core in_maps."""
    packed = _pack_weights(*weights)
    b = x.shape[0]
    b_c = b // n_cores
    b_h = b_c // 2
    in_maps = []
    for c in range(n_cores):
        xc = x[c * b_c:(c + 1) * b_c]
        uc = u[c * b_c:(c + 1) * b_c]
        x2 = np.empty((128, b_h), np.float16)
        x2[0:64] = xc[:b_h].T
        x2[64:128] = xc[b_h:].T
        u6 = np.empty((6, b_h), np.int32)
        u6[0:3] = uc[:b_h]
        u6[3:6] = uc[b_h:]
        in_maps.append({"x2": x2, "u6": u6, **packed})
    return in_maps


def unpack_outputs(results, n_cores=N_CORES):
    b_h = results[0]["yT"].shape[1]
    b_c = 2 * b_h
    y = np.empty((n_cores * b_c, OUT), np.float32)
    for c in range(n_cores):
        yt = results[c]["yT"]
        y[c * b_c:c * b_c + b_h] = yt[0:32].T
        y[c * b_c + b_h:(c + 1) * b_c] = yt[32:64].T
    return y


def kernel(x, u, w1, b1, w2, b2, w3, b3, w4, b4, w5, b5, w6, b6, w7, b7):
    x = np.asarray(x, np.float32)
    u = np.ascontiguousarray(np.asarray(u, np.int32))
    weights = [np.asarray(t, np.float32) for t in
               (w1, b1, w2, b2, w3, b3, w4, b4, w5, b5, w6, b6, w7, b7)]

    nc = _get_nc()
    in_maps = make_in_maps(x, u, weights)
    res = run_bass_kernel_spmd(nc, in_maps, core_ids=list(range(N_CORES)))
    return unpack_outputs(res.results)



# revision 6
# speedup vs baseline: 2.5619x; 2.5619x over previous
"""Trainium2 Bass kernel for the 3-expert MoE routing MLP.

Reference computation (B=1M rows):
    y1  = tanh(x @ w1 - b1)                     # [B, 8]
    h_k = sigmoid(y1 @ wa_k - ba_k)             # [B, 16] for experts k=0,1,2
    e_k = h_k @ wb_k - bb_k                     # [B, 32]
    y   = e_{u[b]}  per row b

Strategy (pure data parallel over 8 cores, ~125000 rows/core):

  * The HOST routes: each core's rows are stably partitioned by expert id
    into 3 segments padded to N_G rows (N_G = 42000 for the seed-0 input,
    0.8% pad).  The device then runs only the SELECTED expert per row as
    dense matmuls -- no masking, no onehot, no u upload -- and the host
    inverts the permutation on unpack.

  * x is shipped as float8_e3m4 (4 mantissa bits), scaled by XS=4 with
    w1 scaled by WS=16 (both folded out via the ACT scale), which halves
    input DMA vs fp16 at ~5e-3 final rel err.  Output is fp16.

  * Per 8*cs-row block (cs=500 free cols; 250 for the segment-tail block):
      - 4 trunk matmuls (fp8e3, M=32 at partition bases 0/32 -- engine APs
        may only start at partition 0/32/64) fill two PSUM tiles
        S_a/S_b[64, cs]; lhsT cols 16-31 are zero so gap rows are zeroed.
      - 2 ACT tanh(S/64 - b1) -> T_y[0:64] and T_y[64:128] fp16.  T_y
        column t holds y1 of 8 rows: partition 32c+8h+f =
        row (8cs*blk + 2(c*cs+t) + h), f the y1 feature.
      - mmH: lhsT [128,128] block-diag wa_k (8 slots x 16 hidden = M 128),
        ONE ACT sigmoid(+ -ba_k) -> T_g[128, cs] fp16.
      - 2 mmF: lhsT [64,128] block-diag wb_k (4 slots x 32 out) over
        T_g[0:64] and T_g[64:128] (weights duplicated at partitions 64-127
        so tile_position rows match), -> 2 PSUM tiles [128, cs].
      - 2 DVE tensor_scalar subtract bb_k: PSUM f32 -> out fp16.
    PE: 7*cs cycles per 8*cs rows = 0.875 cyc/row (vs 1.5 for dense-masked).

  * DMA per core: 16.1 MB in (e3m4) + 8.1 MB out (fp16) -- the roofline.
"""

import math

import numpy as np
import ml_dtypes

import concourse.bass as bass
import concourse.tile as tile
from concourse import mybir
from concourse.bass_utils import run_bass_kernel_spmd

F32 = mybir.dt.float32
F16 = mybir.dt.float16
F8 = mybir.dt.float8e3
E3 = ml_dtypes.float8_e3m4

N_CORES = 8
B = 1_000_000
IN = 64
OUT = 32
B_C = B // N_CORES          # rows per core
N_G_MIN = 42000             # default per-expert segment size (pad target)
XS = 4.0                    # x pre-scale for e3m4 quantization
WS = 16.0                   # w1 pre-scale for e3m4 quantization
E3MAX = 15.5                # largest finite float8_e3m4

# module knobs for the test harness (kernel() itself never reads files)
_TRACE = False
_LAST_RES = None


def _blocks(n_g):
    """Per-segment block list: [(expert, cs)] with 8*cs rows per block."""
    assert n_g % 2000 == 0
    out = []
    for k in range(3):
        rem = n_g
        while rem >= 4000:
            out.append((k, 500))
            rem -= 4000
        if rem:
            assert rem == 2000
            out.append((k, 250))
    return out


def _binfo(n_g):
    """[(expert, cs, x2_col0, y_col0)] for every block, in emission order."""
    info = []
    x0 = y0 = 0
    for k, cs in _blocks(n_g):
        info.append((k, cs, x0, y0))
        x0 += 4 * cs
        y0 += 2 * cs
    return info


def _pack_weights(w1, b1, w2, b2, w3, b3, w4, b4, w5, b5, w6, b6, w7, b7):
    f32 = np.float32
    wa_list = [w2, w4, w6]
    ba_list = [b2, b4, b6]
    wb_list = [w3, w5, w7]
    bb_list = [b3, b5, b7]

    # trunk lhsT [128, 32] e3m4: cols 0-7 even-row w1, 8-15 odd-row, 16-31 zero
    wa8 = np.zeros((128, 32), f32)
    wa8[0:64, 0:8] = WS * w1
    wa8[64:128, 8:16] = WS * w1
    wa8 = np.clip(wa8, -E3MAX, E3MAX).astype(E3)

    # mmH lhsT [128, 128] per expert: row 32c+8h+f -> col block of slot 2c+h
    wh16 = np.zeros((128, 3 * 128), np.float16)
    for k in range(3):
        for s in range(8):
            c, h = s // 2, s % 2
            r0 = 32 * c + 8 * h
            wh16[r0:r0 + 8, 128 * k + 16 * s:128 * k + 16 * s + 16] = \
                wa_list[k].astype(np.float16)

    # mmF lhsT [64, 128] per expert: row 16a+j -> col block of slot a;
    # duplicated at partitions 64-127 for the second (upper-half) matmul.
    wf16 = np.zeros((128, 3 * 128), np.float16)
    for k in range(3):
        for a in range(4):
            blk = wb_list[k].astype(np.float16)
            wf16[16 * a:16 * a + 16, 128 * k + 32 * a:128 * k + 32 * a + 32] = blk
            wf16[64 + 16 * a:64 + 16 * a + 16,
                 128 * k + 32 * a:128 * k + 32 * a + 32] = blk

    # biases [128, 7] f32: col0 trunk -b1; col 1+k mmH -ba_k; col 4+k mmF bb_k
    bp = np.zeros((128, 7), f32)
    for s in range(8):
        c, h = s // 2, s % 2
        bp[32 * c + 8 * h:32 * c + 8 * h + 8, 0] = -b1
    for k in range(3):
        for s in range(8):
            bp[16 * s:16 * s + 16, 1 + k] = -ba_list[k]
        for a in range(4):
            bp[32 * a:32 * a + 32, 4 + k] = bb_list[k]
    return dict(wa8=wa8, wh16=wh16, wf16=wf16, bp=bp)


def _split_multi_waits(nc):
    """Walrus codegen allows one sync-wait per instruction; hoist extra
    waits onto same-engine NoOps inserted just before the instruction."""
    n = 0
    for fn in nc.m.functions:
        for blk in fn.blocks:
            out = []
            for ins in blk.instructions:
                si = ins.sync_info
                if si is not None and len(si.on_wait) > 1:
                    waits = list(si.on_wait)
                    for j, w in enumerate(waits[:-1]):
                        nop = mybir.InstNoOp(name=f"{ins.name}-wsplit{j}")
                        nop.engine = ins.engine
                        nop.sync_info = mybir.SyncInfo(on_wait=[w],
                                                       on_update=[])
                        nc.register_instruction(nop)
                        out.append(nop)
                        n += 1
                    si.on_wait = [waits[-1]]
                out.append(ins)
            blk.instructions[:] = out
    return n


def build_nc(n_g=N_G_MIN):
    nc = bass.Bass("TRN2", target_bir_lowering=False, debug=False)

    R = 3 * n_g                 # padded rows per core
    XC = R // 2                 # x2 columns
    YC = R // 4                 # yT columns

    x2_d = nc.dram_tensor("x2", [128, XC], F8, kind="ExternalInput").ap()
    wa_d = nc.dram_tensor("wa8", [128, 32], F8, kind="ExternalInput").ap()
    wh_d = nc.dram_tensor("wh16", [128, 384], F16, kind="ExternalInput").ap()
    wf_d = nc.dram_tensor("wf16", [128, 384], F16, kind="ExternalInput").ap()
    bp_d = nc.dram_tensor("bp", [128, 7], F32, kind="ExternalInput").ap()
    yt_d = nc.dram_tensor("yT", [128, YC], F16, kind="ExternalOutput").ap()

    TANH = mybir.ActivationFunctionType.Tanh
    SIG = mybir.ActivationFunctionType.Sigmoid
    SUB = mybir.AluOpType.subtract

    info = _binfo(n_g)
    nb = len(info)

    with tile.TileContext(nc) as tc:
        with (
            tc.tile_pool(name="const", bufs=1) as cpool,
            tc.tile_pool(name="xin", bufs=3) as xpool,
            tc.tile_pool(name="ty", bufs=3) as typ,
            tc.tile_pool(name="tg", bufs=3) as tgp,
            tc.tile_pool(name="outp", bufs=3) as opool,
            tc.tile_pool(name="st", bufs=4, space="PSUM") as stp,
            tc.tile_pool(name="sh", bufs=2, space="PSUM") as shp,
            tc.tile_pool(name="pf", bufs=2, space="PSUM") as pfp,
        ):
            wa = cpool.tile([128, 32], F8)
            nc.sync.dma_start(wa[:], wa_d)
            wh = cpool.tile([128, 384], F16)
            nc.sync.dma_start(wh[:], wh_d)
            wf = cpool.tile([128, 384], F16)
            nc.sync.dma_start(wf[:], wf_d)
            bp = cpool.tile([128, 7], F32)
            nc.sync.dma_start(bp[:], bp_d)

            xt = {}
            # prefetch the first three x tiles before the PE warmup so the
            # DMAs overlap it
            for s in range(min(3, nb)):
                k, cs, x0, y0 = info[s]
                xt[s] = xpool.tile([128, 2000], F8, name="xt")
                nc.sync.dma_start(xt[s][:, :4 * cs], x2_d[:, x0:x0 + 4 * cs])

            # HAM warmup: dependency-free back-to-back matmuls so the PE
            # clock gate opens before the main loop.
            for _ in range(20):
                wps = stp.tile([64, 500], F32, name="warm", tag="st")
                nc.tensor.matmul(wps[0:16, 0:384], wh[:, 0:16], wh[:],
                                 start=True, stop=True)

            ty = {}
            tg = {}
            # Software-pipelined emission: per step s the PE does
            # trunk(s), mmF(s-2), mmH(s-1) so every PE wait is
            # pre-satisfied and the PE stream stays dense.
            for s in range(nb + 2):
                if s < nb:
                    k, cs, x0, y0 = info[s]
                    if s + 3 < nb:
                        k3, cs3, x03, _ = info[s + 3]
                        xt[s + 3] = xpool.tile([128, 2000], F8, name="xt")
                        nc.sync.dma_start(xt[s + 3][:, :4 * cs3],
                                          x2_d[:, x03:x03 + 4 * cs3])
                    sa = stp.tile([64, 500], F32, name="sa", tag="st")
                    sb = stp.tile([64, 500], F32, name="sb", tag="st")
                    for c in range(4):
                        dst = (sa if c < 2 else sb)
                        p0 = 32 * (c % 2)
                        nc.tensor.matmul(dst[p0:p0 + 32, :cs], wa[:],
                                         xt[s][:, c * cs:(c + 1) * cs],
                                         start=True, stop=True)
                    ty[s] = typ.tile([128, 500], F16, name="ty")
                    nc.scalar.activation(ty[s][0:64, :cs], sa[:, :cs], TANH,
                                         bias=bp[0:64, 0:1],
                                         scale=1.0 / (XS * WS))
                    nc.scalar.activation(ty[s][64:128, :cs], sb[:, :cs], TANH,
                                         bias=bp[64:128, 0:1],
                                         scale=1.0 / (XS * WS))
                    del sa, sb
                if 0 <= s - 2:
                    b = s - 2
                    k, cs, x0, y0 = info[b]
                    ot = opool.tile([128, 1000], F16, name="ot")
                    pa = pfp.tile([128, 500], F32, name="pa", tag="pf")
                    nc.tensor.matmul(pa[:, :cs], wf[0:64, 128 * k:128 * (k + 1)],
                                     tg[b][0:64, :cs], start=True, stop=True)
                    nc.vector.tensor_scalar(ot[:, 0:cs], pa[:, :cs],
                                            bp[:, 4 + k:5 + k], None, SUB)
                    pb = pfp.tile([128, 500], F32, name="pb", tag="pf")
                    nc.tensor.matmul(pb[:, :cs], wf[64:128, 128 * k:128 * (k + 1)],
                                     tg[b][64:128, :cs], start=True, stop=True)
                    nc.vector.tensor_scalar(ot[:, cs:2 * cs], pb[:, :cs],
                                            bp[:, 4 + k:5 + k], None, SUB)
                    nc.gpsimd.dma_start(yt_d[:, y0:y0 + 2 * cs], ot[:, :2 * cs])
                    del tg[b], pa, pb, ot
                if 0 <= s - 1 < nb:
                    b = s - 1
                    k, cs, x0, y0 = info[b]
                    sh = shp.tile([128, 500], F32, name="sh", tag="sh")
                    nc.tensor.matmul(sh[:, :cs], wh[:, 128 * k:128 * (k + 1)],
                                     ty[b][:, :cs], start=True, stop=True)
                    tg[b] = tgp.tile([128, 500], F16, name="tg")
                    nc.scalar.activation(tg[b][:, :cs], sh[:, :cs], SIG,
                                         bias=bp[:, 1 + k:2 + k], scale=1.0)
                    del ty[b], sh

    _split_multi_waits(nc)
    return nc


_NC_CACHE = {}


def _get_nc(n_g=N_G_MIN):
    if n_g not in _NC_CACHE:
        _NC_CACHE[n_g] = build_nc(n_g)
    return _NC_CACHE[n_g]


_DECODE_CACHE = {}


def _decode_maps(n_g):
    """Per padded-sorted row r: (yT column, yT partition base) arrays."""
    if n_g in _DECODE_CACHE:
        return _DECODE_CACHE[n_g]
    cols = []
    parts = []
    for k, cs, x0, y0 in _binfo(n_g):
        r = np.arange(8 * cs)
        j = r >> 1
        h = r & 1
        c = j // cs
        t = j % cs
        s = 2 * c + h
        e = s >> 2
        a = s & 3
        cols.append(y0 + e * cs + t)
        parts.append(32 * a)
    m = (np.concatenate(cols).astype(np.int32),
         np.concatenate(parts).astype(np.int32))
    _DECODE_CACHE[n_g] = m
    return m


def _choose_n_g(u):
    mx = 0
    for c in range(N_CORES):
        uc = u[c * B_C:(c + 1) * B_C]
        mx = max(mx, int(np.bincount(uc, minlength=3).max()))
    return max(N_G_MIN, 2000 * math.ceil(mx / 2000))


def kernel(x, u, w1, b1, w2, b2, w3, b3, w4, b4, w5, b5, w6, b6, w7, b7):
    global _LAST_RES
    x = np.ascontiguousarray(np.asarray(x, np.float32))
    u = np.ascontiguousarray(np.asarray(u, np.int32))
    weights = [np.asarray(t, np.float32) for t in
               (w1, b1, w2, b2, w3, b3, w4, b4, w5, b5, w6, b6, w7, b7)]

    n_g = _choose_n_g(u)
    R = 3 * n_g
    nc = _get_nc(n_g)
    packed = _pack_weights(*weights)

    in_maps = []
    idx_all = []
    for c in range(N_CORES):
        xc = x[c * B_C:(c + 1) * B_C]
        uc = u[c * B_C:(c + 1) * B_C]
        idx_k = [np.flatnonzero(uc == k) for k in range(3)]
        idx_all.append(idx_k)
        xs = np.zeros((R, IN), np.float32)
        for k in range(3):
            xs[k * n_g:k * n_g + len(idx_k[k])] = xc[idx_k[k]]
        np.clip(xs, -E3MAX / XS, E3MAX / XS, out=xs)
        xs *= XS
        xq = xs.astype(E3)
        x2 = xq.reshape(R // 2, 2, IN).transpose(1, 2, 0).reshape(128, R // 2)
        in_maps.append({"x2": np.ascontiguousarray(x2), **packed})

    res = run_bass_kernel_spmd(nc, in_maps, core_ids=list(range(N_CORES)),
                               trace=_TRACE)
    _LAST_RES = res

    cols, parts = _decode_maps(n_g)
    gather_p = parts[:, None] + np.arange(OUT, dtype=np.int32)[None, :]
    y = np.empty((B, OUT), np.float32)
    for c in range(N_CORES):
        yt = res.results[c]["yT"]
        ys = yt[gather_p, cols[:, None]].astype(np.float32)
        yc = y[c * B_C:(c + 1) * B_C]
        for k in range(3):
            yc[idx_all[c][k]] = ys[k * n_g:k * n_g + len(idx_all[c][k])]
    return y


# revision 16
# speedup vs baseline: 2.8703x; 1.1204x over previous
"""Trainium2 Bass kernel for the 3-expert MoE routing MLP.

Reference computation (B=1M rows):
    y1  = tanh(x @ w1 - b1)                     # [B, 8]
    h_k = sigmoid(y1 @ wa_k - ba_k)             # [B, 16] for experts k=0,1,2
    e_k = h_k @ wb_k - bb_k                     # [B, 32]
    y   = e_{u[b]}  per row b

Strategy (pure data parallel over 8 cores, ~125000 rows/core):

  * The HOST routes: each core's rows are stably partitioned by expert id
    into 3 segments padded to N_G rows (N_G = 42000 for the seed-0 input,
    0.8% pad).  The device then runs only the SELECTED expert per row as
    dense matmuls -- no masking, no onehot, no u upload -- and the host
    inverts the permutation on unpack.

  * x is shipped as float8_e4m3 scaled by XS=2 (w1 by WS=8, both folded
    out via the ACT scale), which halves input DMA vs fp16, and the trunk
    matmul runs in fp8 DoubleRow mode: two K=128 planes accumulate in one
    pass (effective K=256), so each PSUM column holds FOUR rows and the
    trunk costs 0.25 PE cols/row.  Final rel err ~1e-2 (sim-verified).

  * Per 8*cs-row block (cs=500 free cols; 250 for the segment-tail block):
      - 2 DoubleRow trunk matmuls (M=64, zero-padded cols; DoubleRow dst
        must start at partition 0) fill S_a/S_b[64, cs]: partition
        64h+8s'+f of T_y = y1 feature f of slot s = 4h+s'; slot s of
        column t is row 4cs*h + 4t + s'.
      - 2 ACT tanh(S/16 - b1) -> T_y[0:64] / T_y[64:128] fp16.
      - mmH: lhsT [128,128] block-diag wa_k (8 slots x 16 hidden), ONE ACT
        sigmoid(+ -ba_k) -> T_g[128, cs] fp16 (slot s at partitions 16s+).
      - 2 mmF: lhsT [64,128] block-diag wb_k (4 slots x 32 out) over
        T_g[0:64] and T_g[64:128] (weights duplicated at partitions 64-127
        so tile_position rows match), -> 2 PSUM tiles [128, cs].
      - 2 DVE tensor_scalar subtract bb_k: PSUM f32 -> out fp16.
    PE: ~5*cs cycles per 8*cs rows; ACT 2 ops, DVE 2 ops per block.

  * DMA per core: 8.1 MB in (e4m3) + 8.1 MB out (fp16) -- the roofline.
"""

import math

import numpy as np
import ml_dtypes

import concourse.bass as bass
import concourse.tile as tile
from concourse import mybir
from concourse.bass_utils import run_bass_kernel_spmd

F32 = mybir.dt.float32
F16 = mybir.dt.float16
F8 = mybir.dt.float8e4
E4 = ml_dtypes.float8_e4m3

N_CORES = 8
B = 1_000_000
IN = 64
OUT = 32
B_C = B // N_CORES          # rows per core
N_G_MIN = 42000             # default per-expert segment size (pad target)
XS = 2.0                    # x pre-scale for e4m3 quantization
WS = 8.0                    # w1 pre-scale for e4m3 quantization

# module knobs for the test harness (kernel() itself never reads files)
_TRACE = False
_LAST_RES = None


def _blocks(n_g):
    """Per-segment block list: [(expert, cs)] with 8*cs rows per block."""
    assert n_g % 2000 == 0
    out = []
    for k in range(3):
        rem = n_g
        while rem >= 4000:
            out.append((k, 500))
            rem -= 4000
        if rem:
            assert rem == 2000
            out.append((k, 250))
    return out


def _binfo(n_g):
    """[(expert, cs, x_col0, y_col0)] for every block, in emission order.

    x2 and yT share the column index m = row//4 (x2 holds 2 fp8 planes)."""
    info = []
    m0 = 0
    for k, cs in _blocks(n_g):
        info.append((k, cs, m0, m0))
        m0 += 2 * cs
    return info


def _pack_weights(w1, b1, w2, b2, w3, b3, w4, b4, w5, b5, w6, b6, w7, b7):
    f32 = np.float32
    wa_list = [w2, w4, w6]
    ba_list = [b2, b4, b6]
    wb_list = [w3, w5, w7]
    bb_list = [b3, b5, b7]

    # trunk DoubleRow lhsT [128, 2, 64] e4m3 (cols 32-63 zero so that both
    # M=64 matmuls can target 64-aligned PSUM partition bases):
    #   W[64a+f, i, 8s+g] = WS*w1[f, g] where s = 2i+a, else 0
    wa8 = np.zeros((128, 2, 64), f32)
    for i in range(2):
        for a in range(2):
            s = 2 * i + a
            wa8[64 * a:64 * a + 64, i, 8 * s:8 * s + 8] = WS * w1
    wa8 = wa8.astype(E4)

    # mmH lhsT [128, 128] per expert: T_y row 64h+8s'+f (slot s = 4h+s')
    # -> col 16s+j = wa_k[f, j]; rows 32-63 / 96-127 are zero.
    wh16 = np.zeros((128, 3 * 128), np.float16)
    for k in range(3):
        for s in range(8):
            r0 = 64 * (s // 4) + 8 * (s % 4)
            wh16[r0:r0 + 8, 128 * k + 16 * s:128 * k + 16 * s + 16] = \
                wa_list[k].astype(np.float16)

    # mmF lhsT [64, 128] per expert: row 16a+j -> col block of slot a;
    # duplicated at partitions 64-127 for the second (upper-half) matmul.
    wf16 = np.zeros((128, 3 * 128), np.float16)
    for k in range(3):
        for a in range(4):
            blk = wb_list[k].astype(np.float16)
            wf16[16 * a:16 * a + 16, 128 * k + 32 * a:128 * k + 32 * a + 32] = blk
            wf16[64 + 16 * a:64 + 16 * a + 16,
                 128 * k + 32 * a:128 * k + 32 * a + 32] = blk

    # biases [128, 7] f32: col0 trunk -b1 (rows 64h+8s'+f); col 1+k mmH
    # -ba_k (rows 16s+j); col 4+k mmF bb_k (rows 32a+o)
    bp = np.zeros((128, 7), f32)
    for s in range(8):
        r0 = 64 * (s // 4) + 8 * (s % 4)
        bp[r0:r0 + 8, 0] = -b1
    for k in range(3):
        for s in range(8):
            bp[16 * s:16 * s + 16, 1 + k] = -ba_list[k]
        for a in range(4):
            bp[32 * a:32 * a + 32, 4 + k] = bb_list[k]
    return dict(wa8=wa8, wh16=wh16, wf16=wf16, bp=bp)


def _split_multi_waits(nc):
    """Walrus codegen allows one sync-wait per instruction; hoist extra
    waits onto same-engine NoOps inserted just before the instruction."""
    n = 0
    for fn in nc.m.functions:
        for blk in fn.blocks:
            out = []
            for ins in blk.instructions:
                si = ins.sync_info
                if si is not None and len(si.on_wait) > 1:
                    waits = list(si.on_wait)
                    for j, w in enumerate(waits[:-1]):
                        nop = mybir.InstNoOp(name=f"{ins.name}-wsplit{j}")
                        nop.engine = ins.engine
                        nop.sync_info = mybir.SyncInfo(on_wait=[w],
                                                       on_update=[])
                        nc.register_instruction(nop)
                        out.append(nop)
                        n += 1
                    si.on_wait = [waits[-1]]
                out.append(ins)
            blk.instructions[:] = out
    return n


def build_nc(n_g=N_G_MIN):
    nc = bass.Bass("TRN2", target_bir_lowering=False, debug=False)

    R = 3 * n_g                 # padded rows per core
    MC = R // 4                 # x2 / yT columns

    x2_d = nc.dram_tensor("x2", [128, 2, MC], F8, kind="ExternalInput").ap()
    wa_d = nc.dram_tensor("wa8", [128, 2, 64], F8, kind="ExternalInput").ap()
    wh_d = nc.dram_tensor("wh16", [128, 384], F16, kind="ExternalInput").ap()
    wf_d = nc.dram_tensor("wf16", [128, 384], F16, kind="ExternalInput").ap()
    bp_d = nc.dram_tensor("bp", [128, 7], F32, kind="ExternalInput").ap()
    yt_d = nc.dram_tensor("yT", [128, MC], F16, kind="ExternalOutput").ap()

    TANH = mybir.ActivationFunctionType.Tanh
    SIG = mybir.ActivationFunctionType.Sigmoid
    SUB = mybir.AluOpType.subtract
    DR = mybir.MatmulPerfMode.DoubleRow

    info = _binfo(n_g)
    nb = len(info)

    with tile.TileContext(nc) as tc:
        with (
            tc.tile_pool(name="const", bufs=1) as cpool,
            tc.tile_pool(name="xin", bufs=3) as xpool,
            tc.tile_pool(name="ty", bufs=3) as typ,
            tc.tile_pool(name="tg", bufs=3) as tgp,
            tc.tile_pool(name="outp", bufs=3) as opool,
            tc.tile_pool(name="st", bufs=4, space="PSUM") as stp,
            tc.tile_pool(name="sh", bufs=2, space="PSUM") as shp,
            tc.tile_pool(name="pf", bufs=2, space="PSUM") as pfp,
        ):
            wa = cpool.tile([128, 2, 64], F8)
            nc.sync.dma_start(wa[:], wa_d)
            wh = cpool.tile([128, 384], F16)
            nc.sync.dma_start(wh[:], wh_d)
            wf = cpool.tile([128, 384], F16)
            nc.sync.dma_start(wf[:], wf_d)
            bp = cpool.tile([128, 7], F32)
            nc.sync.dma_start(bp[:], bp_d)

            xt = {}
            # prefetch the first three x tiles before the PE warmup so the
            # DMAs overlap it
            for s in range(min(3, nb)):
                k, cs, m0, y0 = info[s]
                xt[s] = xpool.tile([128, 2, 1000], F8, name="xt")
                nc.sync.dma_start(xt[s][:, :, :2 * cs],
                                  x2_d[:, :, m0:m0 + 2 * cs])

            # HAM warmup: dependency-free back-to-back matmuls so the PE
            # clock gate opens before the main loop.
            for _ in range(20):
                wps = stp.tile([64, 500], F32, name="warm", tag="st")
                nc.tensor.matmul(wps[0:16, 0:384], wh[:, 0:16], wh[:],
                                 start=True, stop=True)

            ty = {}
            tg = {}
            # Software-pipelined emission: per step s the PE does
            # trunk(s), mmF(s-2), mmH(s-1) so every PE wait is
            # pre-satisfied and the PE stream stays dense.
            for s in range(nb + 2):
                if s < nb:
                    k, cs, m0, y0 = info[s]
                    if s + 3 < nb:
                        k3, cs3, m03, _ = info[s + 3]
                        xt[s + 3] = xpool.tile([128, 2, 1000], F8, name="xt")
                        nc.sync.dma_start(xt[s + 3][:, :, :2 * cs3],
                                          x2_d[:, :, m03:m03 + 2 * cs3])
                    sa = stp.tile([64, 500], F32, name="sa", tag="st")
                    sb = stp.tile([64, 500], F32, name="sb", tag="st")
                    nc.tensor.matmul(sa[:, :cs], wa[:], xt[s][:, :, 0:cs],
                                     start=True, stop=True, perf_mode=DR)
                    nc.tensor.matmul(sb[:, :cs], wa[:],
                                     xt[s][:, :, cs:2 * cs],
                                     start=True, stop=True, perf_mode=DR)
                    ty[s] = typ.tile([128, 500], F16, name="ty")
                    nc.scalar.activation(ty[s][0:64, :cs], sa[:, :cs], TANH,
                                         bias=bp[0:64, 0:1],
                                         scale=1.0 / (XS * WS))
                    nc.scalar.activation(ty[s][64:128, :cs], sb[:, :cs], TANH,
                                         bias=bp[64:128, 0:1],
                                         scale=1.0 / (XS * WS))
                    del sa, sb
                if 0 <= s - 2:
                    b = s - 2
                    k, cs, m0, y0 = info[b]
                    ot = opool.tile([128, 1000], F16, name="ot")
                    pa = pfp.tile([128, 500], F32, name="pa", tag="pf")
                    nc.tensor.matmul(pa[:, :cs], wf[0:64, 128 * k:128 * (k + 1)],
                                     tg[b][0:64, :cs], start=True, stop=True)
                    nc.vector.tensor_scalar(ot[:, 0:cs], pa[:, :cs],
                                            bp[:, 4 + k:5 + k], None, SUB)
                    pb = pfp.tile([128, 500], F32, name="pb", tag="pf")
                    nc.tensor.matmul(pb[:, :cs], wf[64:128, 128 * k:128 * (k + 1)],
                                     tg[b][64:128, :cs], start=True, stop=True)
                    nc.vector.tensor_scalar(ot[:, cs:2 * cs], pb[:, :cs],
                                            bp[:, 4 + k:5 + k], None, SUB)
                    nc.gpsimd.dma_start(yt_d[:, y0:y0 + 2 * cs], ot[:, :2 * cs])
                    del tg[b], pa, pb, ot
                if 0 <= s - 1 < nb:
                    b = s - 1
                    k, cs, m0, y0 = info[b]
                    sh = shp.tile([128, 500], F32, name="sh", tag="sh")
                    nc.tensor.matmul(sh[:, :cs], wh[:, 128 * k:128 * (k + 1)],
                                     ty[b][:, :cs], start=True, stop=True)
                    tg[b] = tgp.tile([128, 500], F16, name="tg")
                    nc.scalar.activation(tg[b][:, :cs], sh[:, :cs], SIG,
                                         bias=bp[:, 1 + k:2 + k], scale=1.0)
                    del ty[b], sh

    _split_multi_waits(nc)
    return nc


_NC_CACHE = {}


def _get_nc(n_g=N_G_MIN):
    if n_g not in _NC_CACHE:
        _NC_CACHE[n_g] = build_nc(n_g)
    return _NC_CACHE[n_g]


_DECODE_CACHE = {}


def _decode_maps(n_g):
    """Per padded-sorted row r: (yT column, yT partition base) arrays."""
    if n_g in _DECODE_CACHE:
        return _DECODE_CACHE[n_g]
    cols = []
    parts = []
    for k, cs, m0, y0 in _binfo(n_g):
        r = np.arange(8 * cs)
        half = r // (4 * cs)        # which trunk matmul / mmF tile
        w = r % (4 * cs)
        t = w >> 2                  # column within chunk
        a = w & 3                   # slot within half
        cols.append(y0 + half * cs + t)
        parts.append(32 * a)
    m = (np.concatenate(cols).astype(np.int32),
         np.concatenate(parts).astype(np.int32))
    _DECODE_CACHE[n_g] = m
    return m


def _choose_n_g(u):
    mx = 0
    for c in range(N_CORES):
        uc = u[c * B_C:(c + 1) * B_C]
        mx = max(mx, int(np.bincount(uc, minlength=3).max()))
    return max(N_G_MIN, 2000 * math.ceil(mx / 2000))


def kernel(x, u, w1, b1, w2, b2, w3, b3, w4, b4, w5, b5, w6, b6, w7, b7):
    global _LAST_RES
    x = np.ascontiguousarray(np.asarray(x, np.float32))
    u = np.ascontiguousarray(np.asarray(u, np.int32))
    weights = [np.asarray(t, np.float32) for t in
               (w1, b1, w2, b2, w3, b3, w4, b4, w5, b5, w6, b6, w7, b7)]

    n_g = _choose_n_g(u)
    R = 3 * n_g
    nc = _get_nc(n_g)
    packed = _pack_weights(*weights)

    in_maps = []
    idx_all = []
    for c in range(N_CORES):
        xc = x[c * B_C:(c + 1) * B_C]
        uc = u[c * B_C:(c + 1) * B_C]
        idx_k = [np.flatnonzero(uc == k) for k in range(3)]
        idx_all.append(idx_k)
        xs = np.zeros((R, IN), np.float32)
        for k in range(3):
            xs[k * n_g:k * n_g + len(idx_k[k])] = xc[idx_k[k]]
        xs *= XS
        xq = xs.astype(E4)
        # x2[64a+f, i, m] = XS * x[4m + 2i + a, f]
        x2 = xq.reshape(R // 4, 2, 2, IN).transpose(2, 3, 1, 0)
        x2 = x2.reshape(128, 2, R // 4)
        in_maps.append({"x2": np.ascontiguousarray(x2), **packed})

    res = run_bass_kernel_spmd(nc, in_maps, core_ids=list(range(N_CORES)),
                               trace=_TRACE)
    _LAST_RES = res

    cols, parts = _decode_maps(n_g)
    gather_p = parts[:, None] + np.arange(OUT, dtype=np.int32)[None, :]
    y = np.empty((B, OUT), np.float32)
    for c in range(N_CORES):
        yt = res.results[c]["yT"]
        ys = yt[gather_p, cols[:, None]].astype(np.float32)
        yc = y[c * B_C:(c + 1) * B_C]
        for k in range(3):
            yc[idx_all[c][k]] = ys[k * n_g:k * n_g + len(idx_all[c][k])]
    return y


# revision 21
# speedup vs baseline: 2.8704x; 1.0000x over previous
"""Trainium2 Bass kernel for the 3-expert MoE routing MLP.

Reference computation (B=1M rows):
    y1  = tanh(x @ w1 - b1)                     # [B, 8]
    h_k = sigmoid(y1 @ wa_k - ba_k)             # [B, 16] for experts k=0,1,2
    e_k = h_k @ wb_k - bb_k                     # [B, 32]
    y   = e_{u[b]}  per row b

Strategy (pure data parallel over 8 cores, ~125000 rows/core):

  * The HOST routes: each core's rows are stably partitioned by expert id
    into 3 segments padded to N_G rows (N_G = 42000 for the seed-0 input,
    0.8% pad).  The device then runs only the SELECTED expert per row as
    dense matmuls -- no masking, no onehot, no u upload -- and the host
    inverts the permutation on unpack.

  * x is shipped as float8_e4m3 scaled by XS=2 (w1 by WS=8, both folded
    out via the ACT scale), which halves input DMA vs fp16, and the trunk
    matmul runs in fp8 DoubleRow mode: two K=128 planes accumulate in one
    pass (effective K=256), so each PSUM column holds FOUR rows and the
    trunk costs 0.25 PE cols/row.  Final rel err ~1e-2 (sim-verified).

  * Per 8*cs-row block (cs=500 free cols; 250 for the segment-tail block):
      - 2 DoubleRow trunk matmuls (M=64, zero-padded cols; DoubleRow dst
        must start at partition 0) fill S_a/S_b[64, cs]: partition
        64h+8s'+f of T_y = y1 feature f of slot s = 4h+s'; slot s of
        column t is row 4cs*h + 4t + s'.
      - 2 ACT tanh(S/16 - b1) -> T_y[0:64] / T_y[64:128] fp16.
      - mmH: lhsT [128,128] block-diag wa_k (8 slots x 16 hidden), ONE ACT
        sigmoid(+ -ba_k) -> T_g[128, cs] fp16 (slot s at partitions 16s+).
      - 2 mmF: lhsT [64,128] block-diag wb_k (4 slots x 32 out) over
        T_g[0:64] and T_g[64:128] (weights duplicated at partitions 64-127
        so tile_position rows match), -> 2 PSUM tiles [128, cs].
      - 2 DVE tensor_scalar subtract bb_k: PSUM f32 -> out fp16.
    PE: ~5*cs cycles per 8*cs rows; ACT 2 ops, DVE 2 ops per block.

  * DMA per core: 8.1 MB in (e4m3) + 8.1 MB out (fp16) -- the roofline.
"""

import math

import numpy as np
import ml_dtypes

import concourse.bass as bass
import concourse.tile as tile
from concourse import mybir
from concourse.bass_utils import run_bass_kernel_spmd

F32 = mybir.dt.float32
F16 = mybir.dt.float16
F8 = mybir.dt.float8e4
E4 = ml_dtypes.float8_e4m3

N_CORES = 8
B = 1_000_000
IN = 64
OUT = 32
B_C = B // N_CORES          # rows per core
N_G_MIN = 42000             # default per-expert segment size (pad target)
XS = 2.0                    # x pre-scale for e4m3 quantization
WS = 8.0                    # w1 pre-scale for e4m3 quantization

# module knobs for the test harness (kernel() itself never reads files)
_TRACE = False
_LAST_RES = None


def _blocks(n_g):
    """Per-segment block list: [(expert, cs)] with 8*cs rows per block."""
    assert n_g % 2000 == 0
    out = []
    for k in range(3):
        rem = n_g
        while rem >= 4000:
            out.append((k, 500))
            rem -= 4000
        if rem:
            assert rem == 2000
            out.append((k, 250))
    return out


def _binfo(n_g):
    """[(expert, cs, x_col0, y_col0)] for every block, in emission order.

    x2 and yT share the column index m = row//4 (x2 holds 2 fp8 planes)."""
    info = []
    m0 = 0
    for k, cs in _blocks(n_g):
        info.append((k, cs, m0, m0))
        m0 += 2 * cs
    return info


def _tinfo(n_g):
    """DMA tile plan: pairs of big blocks share one x tile / out tile.

    Returns (tiles, owner) where tiles[t] = (m0, mlen) and owner[s] =
    (t, off, is_last_block_of_tile) for every block s of _binfo."""
    info = _binfo(n_g)
    tiles = []
    owner = []
    s = 0
    while s < len(info):
        k, cs, m0, y0 = info[s]
        if (cs == 500 and s + 1 < len(info) and info[s + 1][1] == 500
                and info[s + 1][0] == k):
            tiles.append((m0, 4 * cs))
            owner.append((len(tiles) - 1, 0, False))
            owner.append((len(tiles) - 1, 2 * cs, True))
            s += 2
        else:
            tiles.append((m0, 2 * cs))
            owner.append((len(tiles) - 1, 0, True))
            s += 1
    return tiles, owner


def _pack_weights(w1, b1, w2, b2, w3, b3, w4, b4, w5, b5, w6, b6, w7, b7):
    f32 = np.float32
    wa_list = [w2, w4, w6]
    ba_list = [b2, b4, b6]
    wb_list = [w3, w5, w7]
    bb_list = [b3, b5, b7]

    # trunk DoubleRow lhsT [128, 2, 64] e4m3 (cols 32-63 zero so that both
    # M=64 matmuls can target 64-aligned PSUM partition bases):
    #   W[64a+f, i, 8s+g] = WS*w1[f, g] where s = 2i+a, else 0
    wa8 = np.zeros((128, 2, 64), f32)
    for i in range(2):
        for a in range(2):
            s = 2 * i + a
            wa8[64 * a:64 * a + 64, i, 8 * s:8 * s + 8] = WS * w1
    wa8 = wa8.astype(E4)

    # mmH lhsT [128, 128] per expert: T_y row 64h+8s'+f (slot s = 4h+s')
    # -> col 16s+j = wa_k[f, j]; rows 32-63 / 96-127 are zero.
    wh16 = np.zeros((128, 3 * 128), np.float16)
    for k in range(3):
        for s in range(8):
            r0 = 64 * (s // 4) + 8 * (s % 4)
            wh16[r0:r0 + 8, 128 * k + 16 * s:128 * k + 16 * s + 16] = \
                wa_list[k].astype(np.float16)

    # mmF lhsT [64, 128] per expert: row 16a+j -> col block of slot a;
    # duplicated at partitions 64-127 for the second (upper-half) matmul.
    wf16 = np.zeros((128, 3 * 128), np.float16)
    for k in range(3):
        for a in range(4):
            blk = wb_list[k].astype(np.float16)
            wf16[16 * a:16 * a + 16, 128 * k + 32 * a:128 * k + 32 * a + 32] = blk
            wf16[64 + 16 * a:64 + 16 * a + 16,
                 128 * k + 32 * a:128 * k + 32 * a + 32] = blk

    # biases [128, 7] f32: col0 trunk -b1 (rows 64h+8s'+f); col 1+k mmH
    # -ba_k (rows 16s+j); col 4+k mmF bb_k (rows 32a+o)
    bp = np.zeros((128, 7), f32)
    for s in range(8):
        r0 = 64 * (s // 4) + 8 * (s % 4)
        bp[r0:r0 + 8, 0] = -b1
    for k in range(3):
        for s in range(8):
            bp[16 * s:16 * s + 16, 1 + k] = -ba_list[k]
        for a in range(4):
            bp[32 * a:32 * a + 32, 4 + k] = bb_list[k]
    return dict(wa8=wa8, wh16=wh16, wf16=wf16, bp=bp)


def _split_multi_waits(nc):
    """Walrus codegen allows one sync-wait per instruction; hoist extra
    waits onto same-engine NoOps inserted just before the instruction."""
    n = 0
    for fn in nc.m.functions:
        for blk in fn.blocks:
            out = []
            for ins in blk.instructions:
                si = ins.sync_info
                if si is not None and len(si.on_wait) > 1:
                    waits = list(si.on_wait)
                    for j, w in enumerate(waits[:-1]):
                        nop = mybir.InstNoOp(name=f"{ins.name}-wsplit{j}")
                        nop.engine = ins.engine
                        nop.sync_info = mybir.SyncInfo(on_wait=[w],
                                                       on_update=[])
                        nc.register_instruction(nop)
                        out.append(nop)
                        n += 1
                    si.on_wait = [waits[-1]]
                out.append(ins)
            blk.instructions[:] = out
    return n


def build_nc(n_g=N_G_MIN):
    nc = bass.Bass("TRN2", target_bir_lowering=False, debug=False)

    R = 3 * n_g                 # padded rows per core
    MC = R // 4                 # x2 / yT columns

    x2_d = nc.dram_tensor("x2", [128, 2, MC], F8, kind="ExternalInput").ap()
    wa_d = nc.dram_tensor("wa8", [128, 2, 64], F8, kind="ExternalInput").ap()
    wh_d = nc.dram_tensor("wh16", [128, 384], F16, kind="ExternalInput").ap()
    wf_d = nc.dram_tensor("wf16", [128, 384], F16, kind="ExternalInput").ap()
    bp_d = nc.dram_tensor("bp", [128, 7], F32, kind="ExternalInput").ap()
    yt_d = nc.dram_tensor("yT", [128, MC], F16, kind="ExternalOutput").ap()

    TANH = mybir.ActivationFunctionType.Tanh
    SIG = mybir.ActivationFunctionType.Sigmoid
    SUB = mybir.AluOpType.subtract
    DR = mybir.MatmulPerfMode.DoubleRow

    info = _binfo(n_g)
    tiles, owner = _tinfo(n_g)
    nb = len(info)
    PREFETCH = 6                # blocks of x-tile lookahead

    def xtile_for(xts, b):
        """Issue the x-tile DMA owning block b if not already issued."""
        t, off, last = owner[b]
        if t not in xts:
            m0, mlen = tiles[t]
            xts[t] = xpool.tile([128, 2, 2000], F8, name="xt")
            nc.sync.dma_start(xts[t][:, :, :mlen], x2_d[:, :, m0:m0 + mlen])
        return xts[t]

    with tile.TileContext(nc) as tc:
        with (
            tc.tile_pool(name="const", bufs=1) as cpool,
            tc.tile_pool(name="xin", bufs=5) as xpool,
            tc.tile_pool(name="ty", bufs=3) as typ,
            tc.tile_pool(name="tg", bufs=3) as tgp,
            tc.tile_pool(name="outp", bufs=3) as opool,
            tc.tile_pool(name="st", bufs=4, space="PSUM") as stp,
            tc.tile_pool(name="sh", bufs=2, space="PSUM") as shp,
            tc.tile_pool(name="pf", bufs=2, space="PSUM") as pfp,
        ):
            wa = cpool.tile([128, 2, 64], F8)
            nc.sync.dma_start(wa[:], wa_d)
            wh = cpool.tile([128, 384], F16)
            nc.sync.dma_start(wh[:], wh_d)
            wf = cpool.tile([128, 384], F16)
            nc.sync.dma_start(wf[:], wf_d)
            bp = cpool.tile([128, 7], F32)
            nc.sync.dma_start(bp[:], bp_d)

            xts = {}
            # prefetch the first x tiles before the PE warmup so the DMAs
            # overlap it
            for s in range(min(PREFETCH, nb)):
                xtile_for(xts, s)

            # HAM warmup: dependency-free back-to-back matmuls so the PE
            # clock gate opens before the main loop.
            for _ in range(20):
                wps = stp.tile([64, 500], F32, name="warm", tag="st")
                nc.tensor.matmul(wps[0:16, 0:384], wh[:, 0:16], wh[:],
                                 start=True, stop=True)

            ty = {}
            tg = {}
            ots = {}
            # Software-pipelined emission: per step s the PE does
            # trunk(s), mmF(s-2), mmH(s-1) so every PE wait is
            # pre-satisfied and the PE stream stays dense.
            for s in range(nb + 2):
                if s < nb:
                    k, cs, m0, y0 = info[s]
                    if s + PREFETCH < nb:
                        xtile_for(xts, s + PREFETCH)
                    xt = xts[owner[s][0]]
                    xo = owner[s][1]
                    sa = stp.tile([64, 500], F32, name="sa", tag="st")
                    sb = stp.tile([64, 500], F32, name="sb", tag="st")
                    nc.tensor.matmul(sa[:, :cs], wa[:],
                                     xt[:, :, xo:xo + cs],
                                     start=True, stop=True, perf_mode=DR)
                    nc.tensor.matmul(sb[:, :cs], wa[:],
                                     xt[:, :, xo + cs:xo + 2 * cs],
                                     start=True, stop=True, perf_mode=DR)
                    ty[s] = typ.tile([128, 500], F16, name="ty")
                    nc.scalar.activation(ty[s][0:64, :cs], sa[:, :cs], TANH,
                                         bias=bp[0:64, 0:1],
                                         scale=1.0 / (XS * WS))
                    nc.scalar.activation(ty[s][64:128, :cs], sb[:, :cs], TANH,
                                         bias=bp[64:128, 0:1],
                                         scale=1.0 / (XS * WS))
                    del sa, sb
                    if owner[s][2]:
                        del xts[owner[s][0]]
                if 0 <= s - 2:
                    b = s - 2
                    k, cs, m0, y0 = info[b]
                    t, oo, last = owner[b]
                    if t not in ots:
                        ots[t] = opool.tile([128, 2000], F16, name="ot")
                    ot = ots[t]
                    pa = pfp.tile([128, 500], F32, name="pa", tag="pf")
                    nc.tensor.matmul(pa[:, :cs], wf[0:64, 128 * k:128 * (k + 1)],
                                     tg[b][0:64, :cs], start=True, stop=True)
                    nc.vector.tensor_scalar(ot[:, oo:oo + cs], pa[:, :cs],
                                            bp[:, 4 + k:5 + k], None, SUB)
                    pb = pfp.tile([128, 500], F32, name="pb", tag="pf")
                    nc.tensor.matmul(pb[:, :cs], wf[64:128, 128 * k:128 * (k + 1)],
                                     tg[b][64:128, :cs], start=True, stop=True)
                    nc.vector.tensor_scalar(ot[:, oo + cs:oo + 2 * cs],
                                            pb[:, :cs],
                                            bp[:, 4 + k:5 + k], None, SUB)
                    if last:
                        tm0, tmlen = tiles[t]
                        nc.gpsimd.dma_start(yt_d[:, tm0:tm0 + tmlen],
                                            ot[:, :tmlen])
                        del ots[t]
                    del tg[b], pa, pb
                if 0 <= s - 1 < nb:
                    b = s - 1
                    k, cs, m0, y0 = info[b]
                    sh = shp.tile([128, 500], F32, name="sh", tag="sh")
                    nc.tensor.matmul(sh[:, :cs], wh[:, 128 * k:128 * (k + 1)],
                                     ty[b][:, :cs], start=True, stop=True)
                    tg[b] = tgp.tile([128, 500], F16, name="tg")
                    nc.scalar.activation(tg[b][:, :cs], sh[:, :cs], SIG,
                                         bias=bp[:, 1 + k:2 + k], scale=1.0)
                    del ty[b], sh

    _split_multi_waits(nc)
    return nc


_NC_CACHE = {}


def _get_nc(n_g=N_G_MIN):
    if n_g not in _NC_CACHE:
        _NC_CACHE[n_g] = build_nc(n_g)
    return _NC_CACHE[n_g]


_DECODE_CACHE = {}


def _decode_maps(n_g):
    """Per padded-sorted row r: (yT column, yT partition base) arrays."""
    if n_g in _DECODE_CACHE:
        return _DECODE_CACHE[n_g]
    cols = []
    parts = []
    for k, cs, m0, y0 in _binfo(n_g):
        r = np.arange(8 * cs)
        half = r // (4 * cs)        # which trunk matmul / mmF tile
        w = r % (4 * cs)
        t = w >> 2                  # column within chunk
        a = w & 3                   # slot within half
        cols.append(y0 + half * cs + t)
        parts.append(32 * a)
    m = (np.concatenate(cols).astype(np.int32),
         np.concatenate(parts).astype(np.int32))
    _DECODE_CACHE[n_g] = m
    return m


def _choose_n_g(u):
    mx = 0
    for c in range(N_CORES):
        uc = u[c * B_C:(c + 1) * B_C]
        mx = max(mx, int(np.bincount(uc, minlength=3).max()))
    return max(N_G_MIN, 2000 * math.ceil(mx / 2000))


def kernel(x, u, w1, b1, w2, b2, w3, b3, w4, b4, w5, b5, w6, b6, w7, b7):
    global _LAST_RES
    x = np.ascontiguousarray(np.asarray(x, np.float32))
    u = np.ascontiguousarray(np.asarray(u, np.int32))
    weights = [np.asarray(t, np.float32) for t in
               (w1, b1, w2, b2, w3, b3, w4, b4, w5, b5, w6, b6, w7, b7)]

    n_g = _choose_n_g(u)
    R = 3 * n_g
    nc = _get_nc(n_g)
    packed = _pack_weights(*weights)

    in_maps = []
    idx_all = []
    for c in range(N_CORES):
        xc = x[c * B_C:(c + 1) * B_C]
        uc = u[c * B_C:(c + 1) * B_C]
        idx_k = [np.flatnonzero(uc == k) for k in range(3)]
        idx_all.append(idx_k)
        xs = np.zeros((R, IN), np.float32)
        for k in range(3):
            xs[k * n_g:k * n_g + len(idx_k[k])] = xc[idx_k[k]]
        xs *= XS
        xq = xs.astype(E4)
        # x2[64a+f, i, m] = XS * x[4m + 2i + a, f]
        x2 = xq.reshape(R // 4, 2, 2, IN).transpose(2, 3, 1, 0)
        x2 = x2.reshape(128, 2, R // 4)
        in_maps.append({"x2": np.ascontiguousarray(x2), **packed})

    res = run_bass_kernel_spmd(nc, in_maps, core_ids=list(range(N_CORES)),
                               trace=_TRACE)
    _LAST_RES = res

    cols, parts = _decode_maps(n_g)
    gather_p = parts[:, None] + np.arange(OUT, dtype=np.int32)[None, :]
    y = np.empty((B, OUT), np.float32)
    for c in range(N_CORES):
        yt = res.results[c]["yT"]
        ys = yt[gather_p, cols[:, None]].astype(np.float32)
        yc = y[c * B_C:(c + 1) * B_C]
        for k in range(3):
            yc[idx_all[c][k]] = ys[k * n_g:k * n_g + len(idx_all[c][k])]
    return y


# revision 23
# speedup vs baseline: 2.9352x; 1.0226x over previous
"""Trainium2 Bass kernel for the 3-expert MoE routing MLP.

Reference computation (B=1M rows):
    y1  = tanh(x @ w1 - b1)                     # [B, 8]
    h_k = sigmoid(y1 @ wa_k - ba_k)             # [B, 16] for experts k=0,1,2
    e_k = h_k @ wb_k - bb_k                     # [B, 32]
    y   = e_{u[b]}  per row b

Strategy (pure data parallel over 8 cores, ~125000 rows/core):

  * The HOST routes: each core's rows are stably partitioned by expert id
    into 3 segments padded to N_G rows (N_G = 42000 for the seed-0 input,
    0.8% pad).  The device then runs only the SELECTED expert per row as
    dense matmuls -- no masking, no onehot, no u upload -- and the host
    inverts the permutation on unpack.

  * x is shipped as float8_e4m3 scaled by XS=2 (w1 by WS=8, both folded
    out via the ACT scale), which halves input DMA vs fp16, and the trunk
    matmul runs in fp8 DoubleRow mode: two K=128 planes accumulate in one
    pass (effective K=256), so each PSUM column holds FOUR rows and the
    trunk costs 0.25 PE cols/row.  Final rel err ~1e-2 (sim-verified).

  * Per 8*cs-row block (cs=500 free cols; 250 for the segment-tail block):
      - 2 DoubleRow trunk matmuls (M=64, zero-padded cols; DoubleRow dst
        must start at partition 0) fill S_a/S_b[64, cs]: partition
        64h+8s'+f of T_y = y1 feature f of slot s = 4h+s'; slot s of
        column t is row 4cs*h + 4t + s'.
      - 2 ACT tanh(S/16 - b1) -> T_y[0:64] / T_y[64:128] fp16.
      - mmH: lhsT [128,128] block-diag wa_k (8 slots x 16 hidden), ONE ACT
        sigmoid(+ -ba_k) -> T_g[128, cs] fp16 (slot s at partitions 16s+).
      - 2 mmF: lhsT [64,128] block-diag wb_k (4 slots x 32 out) over
        T_g[0:64] and T_g[64:128] (weights duplicated at partitions 64-127
        so tile_position rows match), -> 2 PSUM tiles [128, cs].
      - 2 DVE tensor_scalar subtract bb_k: PSUM f32 -> out fp16.
    PE: ~5*cs cycles per 8*cs rows; ACT 2 ops, DVE 2 ops per block.

  * DMA per core: 8.1 MB in (e4m3) + 8.1 MB out (fp16) -- the roofline.
"""

import math

import numpy as np
import ml_dtypes

import concourse.bass as bass
import concourse.tile as tile
from concourse import mybir
from concourse.bass_utils import run_bass_kernel_spmd

F32 = mybir.dt.float32
F16 = mybir.dt.float16
F8 = mybir.dt.float8e4
E4 = ml_dtypes.float8_e4m3

N_CORES = 8
B = 1_000_000
IN = 64
OUT = 32
B_C = B // N_CORES          # rows per core
N_G_MIN = 42000             # default per-expert segment size (pad target)
XS = 2.0                    # x pre-scale for e4m3 quantization
WS = 8.0                    # w1 pre-scale for e4m3 quantization

# module knobs for the test harness (kernel() itself never reads files)
_TRACE = False
_LAST_RES = None


def _blocks(n_g):
    """Per-segment block list: [(expert, cs)] with 8*cs rows per block."""
    assert n_g % 2000 == 0
    out = []
    for k in range(3):
        rem = n_g
        while rem >= 4000:
            out.append((k, 500))
            rem -= 4000
        if rem:
            assert rem == 2000
            out.append((k, 250))
    return out


def _binfo(n_g):
    """[(expert, cs, x_col0, y_col0)] for every block, in emission order.

    x2 and yT share the column index m = row//4 (x2 holds 2 fp8 planes)."""
    info = []
    m0 = 0
    for k, cs in _blocks(n_g):
        info.append((k, cs, m0, m0))
        m0 += 2 * cs
    return info


def _tinfo(n_g):
    """DMA tile plan: pairs of big blocks share one x tile / out tile.

    Returns (tiles, owner) where tiles[t] = (m0, mlen) and owner[s] =
    (t, off, is_last_block_of_tile) for every block s of _binfo."""
    info = _binfo(n_g)
    tiles = []
    owner = []
    s = 0
    while s < len(info):
        k, cs, m0, y0 = info[s]
        if (cs == 500 and s + 1 < len(info) and info[s + 1][1] == 500
                and info[s + 1][0] == k):
            tiles.append((m0, 4 * cs))
            owner.append((len(tiles) - 1, 0, False))
            owner.append((len(tiles) - 1, 2 * cs, True))
            s += 2
        else:
            tiles.append((m0, 2 * cs))
            owner.append((len(tiles) - 1, 0, True))
            s += 1
    return tiles, owner


def _pack_weights(w1, b1, w2, b2, w3, b3, w4, b4, w5, b5, w6, b6, w7, b7):
    f32 = np.float32
    wa_list = [w2, w4, w6]
    ba_list = [b2, b4, b6]
    wb_list = [w3, w5, w7]
    bb_list = [b3, b5, b7]

    # trunk DoubleRow lhsT [128, 2, 64] e4m3 (cols 32-63 zero so that both
    # M=64 matmuls can target 64-aligned PSUM partition bases):
    #   W[64a+f, i, 8s+g] = WS*w1[f, g] where s = 2i+a, else 0
    wa8 = np.zeros((128, 2, 64), f32)
    for i in range(2):
        for a in range(2):
            s = 2 * i + a
            wa8[64 * a:64 * a + 64, i, 8 * s:8 * s + 8] = WS * w1
    wa8 = wa8.astype(E4)

    # mmH lhsT [128, 128] per expert: T_y row 64h+8s'+f (slot s = 4h+s')
    # -> col 16s+j = wa_k[f, j]; rows 32-63 / 96-127 are zero.
    wh16 = np.zeros((128, 3 * 128), np.float16)
    for k in range(3):
        for s in range(8):
            r0 = 64 * (s // 4) + 8 * (s % 4)
            wh16[r0:r0 + 8, 128 * k + 16 * s:128 * k + 16 * s + 16] = \
                wa_list[k].astype(np.float16)

    # mmF lhsT [64, 128] per expert: row 16a+j -> col block of slot a;
    # duplicated at partitions 64-127 for the second (upper-half) matmul.
    wf16 = np.zeros((128, 3 * 128), np.float16)
    for k in range(3):
        for a in range(4):
            blk = wb_list[k].astype(np.float16)
            wf16[16 * a:16 * a + 16, 128 * k + 32 * a:128 * k + 32 * a + 32] = blk
            wf16[64 + 16 * a:64 + 16 * a + 16,
                 128 * k + 32 * a:128 * k + 32 * a + 32] = blk

    # biases [128, 7] f32: col0 trunk -b1 (rows 64h+8s'+f); col 1+k mmH
    # -ba_k (rows 16s+j); col 4+k mmF bb_k (rows 32a+o)
    bp = np.zeros((128, 7), f32)
    for s in range(8):
        r0 = 64 * (s // 4) + 8 * (s % 4)
        bp[r0:r0 + 8, 0] = -b1
    for k in range(3):
        for s in range(8):
            bp[16 * s:16 * s + 16, 1 + k] = -ba_list[k]
        for a in range(4):
            bp[32 * a:32 * a + 32, 4 + k] = bb_list[k]
    return dict(wa8=wa8, wh16=wh16, wf16=wf16, bp=bp)


def _split_multi_waits(nc):
    """Walrus codegen allows one sync-wait per instruction; hoist extra
    waits onto same-engine NoOps inserted just before the instruction."""
    n = 0
    for fn in nc.m.functions:
        for blk in fn.blocks:
            out = []
            for ins in blk.instructions:
                si = ins.sync_info
                if si is not None and len(si.on_wait) > 1:
                    waits = list(si.on_wait)
                    for j, w in enumerate(waits[:-1]):
                        nop = mybir.InstNoOp(name=f"{ins.name}-wsplit{j}")
                        nop.engine = ins.engine
                        nop.sync_info = mybir.SyncInfo(on_wait=[w],
                                                       on_update=[])
                        nc.register_instruction(nop)
                        out.append(nop)
                        n += 1
                    si.on_wait = [waits[-1]]
                out.append(ins)
            blk.instructions[:] = out
    return n


def build_nc(n_g=N_G_MIN):
    nc = bass.Bass("TRN2", target_bir_lowering=False, debug=False)

    R = 3 * n_g                 # padded rows per core
    MC = R // 4                 # x2 / yT columns

    x2_d = nc.dram_tensor("x2", [128, 2, MC], F8, kind="ExternalInput").ap()
    wa_d = nc.dram_tensor("wa8", [128, 2, 64], F8, kind="ExternalInput").ap()
    wh_d = nc.dram_tensor("wh16", [128, 384], F16, kind="ExternalInput").ap()
    wf_d = nc.dram_tensor("wf16", [128, 384], F16, kind="ExternalInput").ap()
    bp_d = nc.dram_tensor("bp", [128, 7], F32, kind="ExternalInput").ap()
    yt_d = nc.dram_tensor("yT", [128, MC], F16, kind="ExternalOutput").ap()

    TANH = mybir.ActivationFunctionType.Tanh
    SIG = mybir.ActivationFunctionType.Sigmoid
    SUB = mybir.AluOpType.subtract
    DR = mybir.MatmulPerfMode.DoubleRow

    info = _binfo(n_g)
    tiles, owner = _tinfo(n_g)
    nb = len(info)
    nt = len(tiles)
    # blocks of each DMA tile / superstep, and their in-tile column offsets
    tblocks = [[] for _ in range(nt)]
    for b, (t, off, last) in enumerate(owner):
        tblocks[t].append((b, off))
    # trunk PSUM column offset per block: second block of a pair goes at
    # 512 so each DoubleRow dst stays inside one PSUM bank
    soff = {b: (0 if off == 0 else 512) for b, (t, off, last) in
            enumerate(owner)}
    PREFETCH = 3                # supersteps of x-tile lookahead

    def xtile_for(xts, t):
        if t not in xts:
            m0, mlen = tiles[t]
            xts[t] = xpool.tile([128, 2, 2000], F8, name="xt")
            nc.sync.dma_start(xts[t][:, :, :mlen], x2_d[:, :, m0:m0 + mlen])
        return xts[t]

    with tile.TileContext(nc) as tc:
        with (
            tc.tile_pool(name="const", bufs=1) as cpool,
            tc.tile_pool(name="xin", bufs=4) as xpool,
            tc.tile_pool(name="ty", bufs=2) as typ,
            tc.tile_pool(name="tg", bufs=3) as tgp,
            tc.tile_pool(name="outp", bufs=3) as opool,
            tc.tile_pool(name="sta", bufs=1, space="PSUM") as stap,
            tc.tile_pool(name="stb", bufs=1, space="PSUM") as stbp,
            tc.tile_pool(name="sh", bufs=2, space="PSUM") as shp,
            tc.tile_pool(name="pf", bufs=2, space="PSUM") as pfp,
        ):
            wa = cpool.tile([128, 2, 64], F8)
            nc.sync.dma_start(wa[:], wa_d)
            wh = cpool.tile([128, 384], F16)
            nc.sync.dma_start(wh[:], wh_d)
            wf = cpool.tile([128, 384], F16)
            nc.sync.dma_start(wf[:], wf_d)
            bp = cpool.tile([128, 7], F32)
            nc.sync.dma_start(bp[:], bp_d)

            xts = {}
            for t in range(min(PREFETCH, nt)):
                xtile_for(xts, t)

            ty2 = {}
            tg = {}
            # Superstep-pipelined emission (superstep = one DMA tile = up
            # to 2 blocks): per step t the PE does trunk(t), mmF(t-2),
            # mmH(t-1) so every PE wait is pre-satisfied.
            for t in range(nt + 2):
                if t < nt:
                    if t + PREFETCH < nt:
                        xtile_for(xts, t + PREFETCH)
                    xt = xts[t]
                    # trunk: 2 DoubleRow matmuls per block into shared
                    # [64, 1024] tiles, then ONE ACT per half-superstep
                    sa = stap.tile([64, 1024], F32, name="sa", tag="sta")
                    sb = stbp.tile([64, 1024], F32, name="sb", tag="stb")
                    wid = 0
                    for b, xo in tblocks[t]:
                        cs = info[b][1]
                        so = soff[b]
                        nc.tensor.matmul(sa[:, so:so + cs], wa[:],
                                         xt[:, :, xo:xo + cs],
                                         start=True, stop=True, perf_mode=DR)
                        nc.tensor.matmul(sb[:, so:so + cs], wa[:],
                                         xt[:, :, xo + cs:xo + 2 * cs],
                                         start=True, stop=True, perf_mode=DR)
                        wid = so + cs
                    ty2[t] = typ.tile([128, 1024], F16, name="ty2")
                    nc.scalar.activation(ty2[t][0:64, :wid], sa[:, :wid],
                                         TANH, bias=bp[0:64, 0:1],
                                         scale=1.0 / (XS * WS))
                    nc.scalar.activation(ty2[t][64:128, :wid], sb[:, :wid],
                                         TANH, bias=bp[64:128, 0:1],
                                         scale=1.0 / (XS * WS))
                    del sa, sb, xts[t]
                if 0 <= t - 2:
                    t2 = t - 2
                    ot = opool.tile([128, 2000], F16, name="ot")
                    for b, oo in tblocks[t2]:
                        k, cs, m0, y0 = info[b]
                        pa = pfp.tile([128, 500], F32, name="pa", tag="pf")
                        nc.tensor.matmul(pa[:, :cs],
                                         wf[0:64, 128 * k:128 * (k + 1)],
                                         tg[b][0:64, :cs],
                                         start=True, stop=True)
                        nc.vector.tensor_scalar(ot[:, oo:oo + cs], pa[:, :cs],
                                                bp[:, 4 + k:5 + k], None, SUB)
                        pb = pfp.tile([128, 500], F32, name="pb", tag="pf")
                        nc.tensor.matmul(pb[:, :cs],
                                         wf[64:128, 128 * k:128 * (k + 1)],
                                         tg[b][64:128, :cs],
                                         start=True, stop=True)
                        nc.vector.tensor_scalar(ot[:, oo + cs:oo + 2 * cs],
                                                pb[:, :cs],
                                                bp[:, 4 + k:5 + k], None, SUB)
                        del tg[b], pa, pb
                    tm0, tmlen = tiles[t2]
                    nc.gpsimd.dma_start(yt_d[:, tm0:tm0 + tmlen],
                                        ot[:, :tmlen])
                    del ot
                if 0 <= t - 1 < nt:
                    t1 = t - 1
                    for b, _ in tblocks[t1]:
                        k, cs, m0, y0 = info[b]
                        so = soff[b]
                        sh = shp.tile([128, 500], F32, name="sh", tag="sh")
                        nc.tensor.matmul(sh[:, :cs],
                                         wh[:, 128 * k:128 * (k + 1)],
                                         ty2[t1][:, so:so + cs],
                                         start=True, stop=True)
                        tg[b] = tgp.tile([128, 500], F16, name="tg")
                        nc.scalar.activation(tg[b][:, :cs], sh[:, :cs], SIG,
                                             bias=bp[:, 1 + k:2 + k],
                                             scale=1.0)
                        del sh
                    del ty2[t1]

    _split_multi_waits(nc)
    return nc


_NC_CACHE = {}


def _get_nc(n_g=N_G_MIN):
    if n_g not in _NC_CACHE:
        _NC_CACHE[n_g] = build_nc(n_g)
    return _NC_CACHE[n_g]


_DECODE_CACHE = {}


def _decode_maps(n_g):
    """Per padded-sorted row r: (yT column, yT partition base) arrays."""
    if n_g in _DECODE_CACHE:
        return _DECODE_CACHE[n_g]
    cols = []
    parts = []
    for k, cs, m0, y0 in _binfo(n_g):
        r = np.arange(8 * cs)
        half = r // (4 * cs)        # which trunk matmul / mmF tile
        w = r % (4 * cs)
        t = w >> 2                  # column within chunk
        a = w & 3                   # slot within half
        cols.append(y0 + half * cs + t)
        parts.append(32 * a)
    m = (np.concatenate(cols).astype(np.int32),
         np.concatenate(parts).astype(np.int32))
    _DECODE_CACHE[n_g] = m
    return m


def _choose_n_g(u):
    mx = 0
    for c in range(N_CORES):
        uc = u[c * B_C:(c + 1) * B_C]
        mx = max(mx, int(np.bincount(uc, minlength=3).max()))
    return max(N_G_MIN, 2000 * math.ceil(mx / 2000))


def kernel(x, u, w1, b1, w2, b2, w3, b3, w4, b4, w5, b5, w6, b6, w7, b7):
    global _LAST_RES
    x = np.ascontiguousarray(np.asarray(x, np.float32))
    u = np.ascontiguousarray(np.asarray(u, np.int32))
    weights = [np.asarray(t, np.float32) for t in
               (w1, b1, w2, b2, w3, b3, w4, b4, w5, b5, w6, b6, w7, b7)]

    n_g = _choose_n_g(u)
    R = 3 * n_g
    nc = _get_nc(n_g)
    packed = _pack_weights(*weights)

    in_maps = []
    idx_all = []
    for c in range(N_CORES):
        xc = x[c * B_C:(c + 1) * B_C]
        uc = u[c * B_C:(c + 1) * B_C]
        idx_k = [np.flatnonzero(uc == k) for k in range(3)]
        idx_all.append(idx_k)
        xs = np.zeros((R, IN), np.float32)
        for k in range(3):
            xs[k * n_g:k * n_g + len(idx_k[k])] = xc[idx_k[k]]
        xs *= XS
        xq = xs.astype(E4)
        # x2[64a+f, i, m] = XS * x[4m + 2i + a, f]
        x2 = xq.reshape(R // 4, 2, 2, IN).transpose(2, 3, 1, 0)
        x2 = x2.reshape(128, 2, R // 4)
        in_maps.append({"x2": np.ascontiguousarray(x2), **packed})

    res = run_bass_kernel_spmd(nc, in_maps, core_ids=list(range(N_CORES)),
                               trace=_TRACE)
    _LAST_RES = res

    cols, parts = _decode_maps(n_g)
    gather_p = parts[:, None] + np.arange(OUT, dtype=np.int32)[None, :]
    y = np.empty((B, OUT), np.float32)
    for c in range(N_CORES):
        yt = res.results[c]["yT"]
        ys = yt[gather_p, cols[:, None]].astype(np.float32)
        yc = y[c * B_C:(c + 1) * B_C]
        for k in range(3):
            yc[idx_all[c][k]] = ys[k * n_g:k * n_g + len(idx_all[c][k])]
    return y


# revision 28
# speedup vs baseline: 3.0201x; 1.0289x over previous
"""Trainium2 Bass kernel for the 3-expert MoE routing MLP.

Reference computation (B=1M rows):
    y1  = tanh(x @ w1 - b1)                     # [B, 8]
    h_k = sigmoid(y1 @ wa_k - ba_k)             # [B, 16] for experts k=0,1,2
    e_k = h_k @ wb_k - bb_k                     # [B, 32]
    y   = e_{u[b]}  per row b

Strategy (pure data parallel over 8 cores, ~125000 rows/core):

  * The HOST routes: each core's rows are stably partitioned by expert id
    into 3 segments padded to N_G rows (N_G = 42000 for the seed-0 input,
    0.8% pad).  The device then runs only the SELECTED expert per row as
    dense matmuls -- no masking, no onehot, no u upload -- and the host
    inverts the permutation on unpack.

  * x is shipped as float8_e4m3 scaled by XS=2 (w1 by WS=8, both folded
    out via the ACT scale), which halves input DMA vs fp16, and the trunk
    matmul runs in fp8 DoubleRow mode: two K=128 planes accumulate in one
    pass (effective K=256), so each PSUM column holds FOUR rows and the
    trunk costs 0.25 PE cols/row.  Final rel err ~1e-2 (sim-verified).

  * Per 8*cs-row block (cs=500 free cols; 250 for the segment-tail block):
      - 2 DoubleRow trunk matmuls (M=64, zero-padded cols; DoubleRow dst
        must start at partition 0) fill S_a/S_b[64, cs]: partition
        64h+8s'+f of T_y = y1 feature f of slot s = 4h+s'; slot s of
        column t is row 4cs*h + 4t + s'.
      - 2 ACT tanh(S/16 - b1) -> T_y[0:64] / T_y[64:128] fp16.
      - mmH: lhsT [128,128] block-diag wa_k (8 slots x 16 hidden), ONE ACT
        sigmoid(+ -ba_k) -> T_g[128, cs] fp16 (slot s at partitions 16s+).
      - 2 mmF: lhsT [64,128] block-diag wb_k (4 slots x 32 out) over
        T_g[0:64] and T_g[64:128] (weights duplicated at partitions 64-127
        so tile_position rows match), -> 2 PSUM tiles [128, cs].
      - 2 DVE tensor_scalar subtract bb_k: PSUM f32 -> out fp16.
    PE: ~5*cs cycles per 8*cs rows; ACT 2 ops, DVE 2 ops per block.

  * DMA per core: 8.1 MB in (e4m3) + 8.1 MB out (fp16) -- the roofline.
"""

import math

import numpy as np
import ml_dtypes

import concourse.bass as bass
import concourse.tile as tile
from concourse import mybir
from concourse.bass_utils import run_bass_kernel_spmd

F32 = mybir.dt.float32
F16 = mybir.dt.float16
F8 = mybir.dt.float8e4
E4 = ml_dtypes.float8_e4m3

N_CORES = 8
B = 1_000_000
IN = 64
OUT = 32
B_C = B // N_CORES          # rows per core
N_G_MIN = 42000             # default per-expert segment size (pad target)
XS = 2.0                    # x pre-scale for e4m3 quantization
WS = 8.0                    # w1 pre-scale for e4m3 quantization

# module knobs for the test harness (kernel() itself never reads files)
_TRACE = False
_LAST_RES = None


def _blocks(n_g):
    """Per-segment block list: [(expert, cs)] with 8*cs rows per block."""
    assert n_g % 2000 == 0
    out = []
    for k in range(3):
        rem = n_g
        while rem >= 4000:
            out.append((k, 500))
            rem -= 4000
        if rem:
            assert rem == 2000
            out.append((k, 250))
    return out


def _binfo(n_g):
    """[(expert, cs, x_col0, y_col0)] for every block, in emission order.

    x2 and yT share the column index m = row//4 (x2 holds 2 fp8 planes)."""
    info = []
    m0 = 0
    for k, cs in _blocks(n_g):
        info.append((k, cs, m0, m0))
        m0 += 2 * cs
    return info


def _tinfo(n_g):
    """DMA tile plan: pairs of big blocks share one x tile / out tile.

    Returns (tiles, owner) where tiles[t] = (m0, mlen) and owner[s] =
    (t, off, is_last_block_of_tile) for every block s of _binfo."""
    info = _binfo(n_g)
    tiles = []
    owner = []
    s = 0
    while s < len(info):
        k, cs, m0, y0 = info[s]
        if (cs == 500 and s + 1 < len(info) and info[s + 1][1] == 500
                and info[s + 1][0] == k):
            tiles.append((m0, 4 * cs))
            owner.append((len(tiles) - 1, 0, False))
            owner.append((len(tiles) - 1, 2 * cs, True))
            s += 2
        else:
            tiles.append((m0, 2 * cs))
            owner.append((len(tiles) - 1, 0, True))
            s += 1
    return tiles, owner


def _pack_weights(w1, b1, w2, b2, w3, b3, w4, b4, w5, b5, w6, b6, w7, b7):
    f32 = np.float32
    wa_list = [w2, w4, w6]
    ba_list = [b2, b4, b6]
    wb_list = [w3, w5, w7]
    bb_list = [b3, b5, b7]

    # trunk DoubleRow lhsT [128, 2, 64] e4m3 (cols 32-63 zero so that both
    # M=64 matmuls can target 64-aligned PSUM partition bases):
    #   W[64a+f, i, 8s+g] = WS*w1[f, g] where s = 2i+a, else 0
    wa8 = np.zeros((128, 2, 64), f32)
    for i in range(2):
        for a in range(2):
            s = 2 * i + a
            wa8[64 * a:64 * a + 64, i, 8 * s:8 * s + 8] = WS * w1
    wa8 = wa8.astype(E4)

    # mmH lhsT [128, 128] per expert: T_y row 64h+8s'+f (slot s = 4h+s')
    # -> col 16s+j = wa_k[f, j]; rows 32-63 / 96-127 are zero.
    wh16 = np.zeros((128, 3 * 128), np.float16)
    for k in range(3):
        for s in range(8):
            r0 = 64 * (s // 4) + 8 * (s % 4)
            wh16[r0:r0 + 8, 128 * k + 16 * s:128 * k + 16 * s + 16] = \
                wa_list[k].astype(np.float16)

    # mmF lhsT [64, 128] per expert: row 16a+j -> col block of slot a;
    # duplicated at partitions 64-127 for the second (upper-half) matmul.
    wf16 = np.zeros((128, 3 * 128), np.float16)
    for k in range(3):
        for a in range(4):
            blk = wb_list[k].astype(np.float16)
            wf16[16 * a:16 * a + 16, 128 * k + 32 * a:128 * k + 32 * a + 32] = blk
            wf16[64 + 16 * a:64 + 16 * a + 16,
                 128 * k + 32 * a:128 * k + 32 * a + 32] = blk

    # biases [128, 10] f32: col0 trunk -b1 (rows 64h+8s'+f); col 1+k mmH
    # -ba_k (rows 16s+j); col 4+k mmF bb_k (rows 32a+o); col 7+k -bb_k
    # (for ACT Identity adds in the drain tail)
    bp = np.zeros((128, 10), f32)
    for s in range(8):
        r0 = 64 * (s // 4) + 8 * (s % 4)
        bp[r0:r0 + 8, 0] = -b1
    for k in range(3):
        for s in range(8):
            bp[16 * s:16 * s + 16, 1 + k] = -ba_list[k]
        for a in range(4):
            bp[32 * a:32 * a + 32, 4 + k] = bb_list[k]
            bp[32 * a:32 * a + 32, 7 + k] = -bb_list[k]
    return dict(wa8=wa8, wh16=wh16, wf16=wf16, bp=bp)


def _split_multi_waits(nc):
    """Walrus codegen allows one sync-wait per instruction; hoist extra
    waits onto same-engine NoOps inserted just before the instruction."""
    n = 0
    for fn in nc.m.functions:
        for blk in fn.blocks:
            out = []
            for ins in blk.instructions:
                si = ins.sync_info
                if si is not None and len(si.on_wait) > 1:
                    waits = list(si.on_wait)
                    for j, w in enumerate(waits[:-1]):
                        nop = mybir.InstNoOp(name=f"{ins.name}-wsplit{j}")
                        nop.engine = ins.engine
                        nop.sync_info = mybir.SyncInfo(on_wait=[w],
                                                       on_update=[])
                        nc.register_instruction(nop)
                        out.append(nop)
                        n += 1
                    si.on_wait = [waits[-1]]
                out.append(ins)
            blk.instructions[:] = out
    return n


def build_nc(n_g=N_G_MIN):
    nc = bass.Bass("TRN2", target_bir_lowering=False, debug=False)

    R = 3 * n_g                 # padded rows per core
    MC = R // 4                 # x2 / yT columns

    x2_d = nc.dram_tensor("x2", [128, 2, MC], F8, kind="ExternalInput").ap()
    wa_d = nc.dram_tensor("wa8", [128, 2, 64], F8, kind="ExternalInput").ap()
    wh_d = nc.dram_tensor("wh16", [128, 384], F16, kind="ExternalInput").ap()
    wf_d = nc.dram_tensor("wf16", [128, 384], F16, kind="ExternalInput").ap()
    bp_d = nc.dram_tensor("bp", [128, 10], F32, kind="ExternalInput").ap()
    yt_d = nc.dram_tensor("yT", [128, MC], F16, kind="ExternalOutput").ap()

    TANH = mybir.ActivationFunctionType.Tanh
    SIG = mybir.ActivationFunctionType.Sigmoid
    SUB = mybir.AluOpType.subtract
    IDENT = mybir.ActivationFunctionType.Identity
    DR = mybir.MatmulPerfMode.DoubleRow

    info = _binfo(n_g)
    tiles, owner = _tinfo(n_g)
    nb = len(info)
    nt = len(tiles)
    # blocks of each DMA tile / superstep, and their in-tile column offsets
    tblocks = [[] for _ in range(nt)]
    for b, (t, off, last) in enumerate(owner):
        tblocks[t].append((b, off))
    # trunk PSUM column offset per block: second block of a pair goes at
    # 512 so each DoubleRow dst stays inside one PSUM bank
    soff = {b: (0 if off == 0 else 512) for b, (t, off, last) in
            enumerate(owner)}
    PREFETCH = 3                # supersteps of x-tile lookahead

    def xtile_for(xts, t):
        if t not in xts:
            m0, mlen = tiles[t]
            xts[t] = xpool.tile([128, 2, 2000], F8, name="xt")
            nc.sync.dma_start(xts[t][:, :, :mlen], x2_d[:, :, m0:m0 + mlen])
        return xts[t]

    with tile.TileContext(nc) as tc:
        with (
            tc.tile_pool(name="const", bufs=1) as cpool,
            tc.tile_pool(name="xin", bufs=4) as xpool,
            tc.tile_pool(name="ty", bufs=2) as typ,
            tc.tile_pool(name="tg", bufs=3) as tgp,
            tc.tile_pool(name="outp", bufs=3) as opool,
            tc.tile_pool(name="sta", bufs=1, space="PSUM") as stap,
            tc.tile_pool(name="stb", bufs=1, space="PSUM") as stbp,
            tc.tile_pool(name="sh", bufs=2, space="PSUM") as shp,
            tc.tile_pool(name="pf", bufs=2, space="PSUM") as pfp,
        ):
            wa = cpool.tile([128, 2, 64], F8)
            nc.sync.dma_start(wa[:], wa_d)
            wh = cpool.tile([128, 384], F16)
            nc.sync.dma_start(wh[:], wh_d)
            wf = cpool.tile([128, 384], F16)
            nc.sync.dma_start(wf[:], wf_d)
            bp = cpool.tile([128, 10], F32)
            nc.sync.dma_start(bp[:], bp_d)

            xts = {}
            for t in range(min(PREFETCH, nt)):
                xtile_for(xts, t)

            # Short HAM/pstate warmup: dependency-free matmuls that run
            # while the first x tiles stream in, so the PE clock is ramped
            # when the main loop starts.
            for _ in range(8):
                wps = shp.tile([128, 500], F32, name="warm", tag="sh")
                nc.tensor.matmul(wps[0:16, 0:384], wh[:, 0:16], wh[:],
                                 start=True, stop=True)

            ty2 = {}
            tg = {}
            # Superstep-pipelined emission (superstep = one DMA tile = up
            # to 2 blocks): per step t the PE does trunk(t), mmF(t-2),
            # mmH(t-1) so every PE wait is pre-satisfied.
            for t in range(nt + 2):
                if t < nt:
                    if t + PREFETCH < nt:
                        xtile_for(xts, t + PREFETCH)
                    xt = xts[t]
                    # trunk: 2 DoubleRow matmuls per block into shared
                    # [64, 1024] tiles, then ONE ACT per half-superstep
                    sa = stap.tile([64, 1024], F32, name="sa", tag="sta")
                    sb = stbp.tile([64, 1024], F32, name="sb", tag="stb")
                    wid = 0
                    for b, xo in tblocks[t]:
                        cs = info[b][1]
                        so = soff[b]
                        nc.tensor.matmul(sa[:, so:so + cs], wa[:],
                                         xt[:, :, xo:xo + cs],
                                         start=True, stop=True, perf_mode=DR)
                        nc.tensor.matmul(sb[:, so:so + cs], wa[:],
                                         xt[:, :, xo + cs:xo + 2 * cs],
                                         start=True, stop=True, perf_mode=DR)
                        wid = so + cs
                    ty2[t] = typ.tile([128, 1024], F16, name="ty2")
                    nc.scalar.activation(ty2[t][0:64, :wid], sa[:, :wid],
                                         TANH, bias=bp[0:64, 0:1],
                                         scale=1.0 / (XS * WS))
                    nc.scalar.activation(ty2[t][64:128, :wid], sb[:, :wid],
                                         TANH, bias=bp[64:128, 0:1],
                                         scale=1.0 / (XS * WS))
                    del sa, sb, xts[t]
                if 0 <= t - 2:
                    t2 = t - 2
                    ot = opool.tile([128, 2000], F16, name="ot")
                    for b, oo in tblocks[t2]:
                        k, cs, m0, y0 = info[b]
                        pa = pfp.tile([128, 500], F32, name="pa", tag="pf")
                        nc.tensor.matmul(pa[:, :cs],
                                         wf[0:64, 128 * k:128 * (k + 1)],
                                         tg[b][0:64, :cs],
                                         start=True, stop=True)
                        if t2 >= nt - 2:
                            # drain tail: ACT is idle here -- run half the
                            # bias-subtracts on it so DVE isn't the pacer
                            nc.scalar.activation(ot[:, oo:oo + cs],
                                                 pa[:, :cs], IDENT,
                                                 bias=bp[:, 7 + k:8 + k],
                                                 scale=1.0)
                        else:
                            nc.vector.tensor_scalar(ot[:, oo:oo + cs],
                                                    pa[:, :cs],
                                                    bp[:, 4 + k:5 + k],
                                                    None, SUB)
                        pb = pfp.tile([128, 500], F32, name="pb", tag="pf")
                        nc.tensor.matmul(pb[:, :cs],
                                         wf[64:128, 128 * k:128 * (k + 1)],
                                         tg[b][64:128, :cs],
                                         start=True, stop=True)
                        nc.vector.tensor_scalar(ot[:, oo + cs:oo + 2 * cs],
                                                pb[:, :cs],
                                                bp[:, 4 + k:5 + k], None, SUB)
                        del tg[b], pa, pb
                    tm0, tmlen = tiles[t2]
                    nc.gpsimd.dma_start(yt_d[:, tm0:tm0 + tmlen],
                                        ot[:, :tmlen])
                    del ot
                if 0 <= t - 1 < nt:
                    t1 = t - 1
                    for b, _ in tblocks[t1]:
                        k, cs, m0, y0 = info[b]
                        so = soff[b]
                        sh = shp.tile([128, 500], F32, name="sh", tag="sh")
                        nc.tensor.matmul(sh[:, :cs],
                                         wh[:, 128 * k:128 * (k + 1)],
                                         ty2[t1][:, so:so + cs],
                                         start=True, stop=True)
                        tg[b] = tgp.tile([128, 500], F16, name="tg")
                        nc.scalar.activation(tg[b][:, :cs], sh[:, :cs], SIG,
                                             bias=bp[:, 1 + k:2 + k],
                                             scale=1.0)
                        del sh
                    del ty2[t1]

    _split_multi_waits(nc)
    return nc


_NC_CACHE = {}


def _get_nc(n_g=N_G_MIN):
    if n_g not in _NC_CACHE:
        _NC_CACHE[n_g] = build_nc(n_g)
    return _NC_CACHE[n_g]


_DECODE_CACHE = {}


def _decode_maps(n_g):
    """Per padded-sorted row r: (yT column, yT partition base) arrays."""
    if n_g in _DECODE_CACHE:
        return _DECODE_CACHE[n_g]
    cols = []
    parts = []
    for k, cs, m0, y0 in _binfo(n_g):
        r = np.arange(8 * cs)
        half = r // (4 * cs)        # which trunk matmul / mmF tile
        w = r % (4 * cs)
        t = w >> 2                  # column within chunk
        a = w & 3                   # slot within half
        cols.append(y0 + half * cs + t)
        parts.append(32 * a)
    m = (np.concatenate(cols).astype(np.int32),
         np.concatenate(parts).astype(np.int32))
    _DECODE_CACHE[n_g] = m
    return m


def _choose_n_g(u):
    mx = 0
    for c in range(N_CORES):
        uc = u[c * B_C:(c + 1) * B_C]
        mx = max(mx, int(np.bincount(uc, minlength=3).max()))
    return max(N_G_MIN, 2000 * math.ceil(mx / 2000))


def kernel(x, u, w1, b1, w2, b2, w3, b3, w4, b4, w5, b5, w6, b6, w7, b7):
    global _LAST_RES
    x = np.ascontiguousarray(np.asarray(x, np.float32))
    u = np.ascontiguousarray(np.asarray(u, np.int32))
    weights = [np.asarray(t, np.float32) for t in
               (w1, b1, w2, b2, w3, b3, w4, b4, w5, b5, w6, b6, w7, b7)]

    n_g = _choose_n_g(u)
    R = 3 * n_g
    nc = _get_nc(n_g)
    packed = _pack_weights(*weights)

    in_maps = []
    idx_all = []
    for c in range(N_CORES):
        xc = x[c * B_C:(c + 1) * B_C]
        uc = u[c * B_C:(c + 1) * B_C]
        idx_k = [np.flatnonzero(uc == k) for k in range(3)]
        idx_all.append(idx_k)
        xs = np.zeros((R, IN), np.float32)
        for k in range(3):
            xs[k * n_g:k * n_g + len(idx_k[k])] = xc[idx_k[k]]
        xs *= XS
        xq = xs.astype(E4)
        # x2[64a+f, i, m] = XS * x[4m + 2i + a, f]
        x2 = xq.reshape(R // 4, 2, 2, IN).transpose(2, 3, 1, 0)
        x2 = x2.reshape(128, 2, R // 4)
        in_maps.append({"x2": np.ascontiguousarray(x2), **packed})

    res = run_bass_kernel_spmd(nc, in_maps, core_ids=list(range(N_CORES)),
                               trace=_TRACE)
    _LAST_RES = res

    cols, parts = _decode_maps(n_g)
    gather_p = parts[:, None] + np.arange(OUT, dtype=np.int32)[None, :]
    y = np.empty((B, OUT), np.float32)
    for c in range(N_CORES):
        yt = res.results[c]["yT"]
        ys = yt[gather_p, cols[:, None]].astype(np.float32)
        yc = y[c * B_C:(c + 1) * B_C]
        for k in range(3):
            yc[idx_all[c][k]] = ys[k * n_g:k * n_g + len(idx_all[c][k])]
    return y
